# revision 1
# baseline (speedup 1.0000x reference)
"""ChirpTextureSynth Trainium2 kernel.

Synthesizes 4096 windowed chirp grains (16384 samples each), scatter-adds
them at per-grain onsets into a 524288-sample signal, L2-normalizes.

Strategy (8 NeuronCores, data-parallel over grains, 512 grains/core):
 - Output accumulator layout: sample s -> (partition p = s % 128, col = s // 128).
   A grain at onset o occupies cols [o//128, o//128 + 129) on all partitions
   (onsets never wrap: o < N_SAMPLES - GRAIN_N).
 - Sine argument in CYCLES: theta(p,c) = f0*phase(t), t = i/SR - D/2,
   i = 128*c + p - (o % 128). theta is low-rank separable in (p, c):
   exp branch  : theta = a*E(p)*F(c) + b,  E(p)=exp(g*p/SR)
   taylor branch (|g| < 0.7): theta = sum_j coeff_j(c) * p^j, j=0..4
 - Range reduction is folded INTO the theta matmul: S piecewise-constant
   partition-segment hint rows (indicator lhsT rows x integer bf16 rhs rows)
   subtract round(theta) per segment, leaving |u| <= ~0.58.  The ACT Sin
   spline (scale=2pi) is accurate to ~3e-4 out to |u|=0.58, so ONE K<=128
   matmul per 3 grains produces ready-to-sin u tiles in PSUM (f32).
   Fallback for extreme chirp slopes (segment residual too big): per-element
   int8 hint plane applied via identity matmul / DVE, as before.
 - ACT Sin(scale=2pi) evaluates sin(2*pi*u) -> bf16 SBUF.
 - Window*amp (bf16 host table WA) applied on DVE: v = s * WA (2x bf16).
 - Scatter: per-grain matmul with identity weights accumulates v into a
   PSUM "strip" bank (512 cols); strips follow onset-sorted grains; DVE
   evacuates each strip into the SBUF accumulator.  Columns left of the
   next strip's base are final and are streamed out to DRAM eagerly.
 - Per-core instruction streams differ (grain offsets are immediates), so the
   program has 8 tc.If(partition_id == c) branches; inputs differ per core.
 - Reduction: ReduceScatter (128x4096 f32) + scalar AllReduce of sum-of-
   squares; each core normalizes and outputs its 1/8 chunk; host reassembles.
"""

import math
from contextlib import ExitStack

import numpy as np

SR = 44100.0
N_SAMPLES = 524288
N_GRAINS = 4096
GRAIN_N = 16384
F0_MIN = 32.7
F0_MAX = 523.25
Q = 12
HOP_LEN = 256
GRAIN_DUR_S = GRAIN_N / SR
N_CORES = 8
GPC = N_GRAINS // N_CORES  # grains per core (512)

ACC_COLS = N_SAMPLES // 128        # 4096
ACC_PAD_COLS = ACC_COLS + 384      # strip overhang room
GCOLS = 129                        # cols per grain tile
BATCH = 9                          # grains per compute batch (3 psum banks)
GPB = 3                           # grains per theta-matmul (387 cols <= 512)
TAYLOR_CUT = 0.7                   # |gamma| below which the poly branch is used
STRIP_COLS = 512
SEG_MARGIN = 0.65                  # max |u| the Sin spline tolerates (~5e-3)
DMA_B = 2                          # batches per input DMA chunk
DMA_PRE = 8                        # chunks interleaved rhs/wa at the start
FLUSH_MIN = 384                    # min final cols before an output flush


def _host_prep(theta_density, theta_slope, f0_freqs_hz, onsets):
    """All host-side precompute. Returns per-core input arrays + metadata."""
    td = float(np.float32(theta_density))
    ts = float(np.float32(theta_slope))
    f0 = np.asarray(f0_freqs_hz, dtype=np.float64)
    on = np.asarray(onsets, dtype=np.int64)

    # per-grain amplitudes (matches reference, f64 is fine vs f32 ref)
    gi = np.arange(N_GRAINS, dtype=np.float64)
    offset = 0.25 * td + 0.75 * td * td
    sig_op = (1.0 - td) * N_GRAINS * (gi / N_GRAINS - offset)
    amps = 0.5 * (1.0 - np.tanh(sig_op))  # = 1 - sigmoid(2*sig_op), stable
    amps = amps / amps.max()
    A = amps / np.sqrt(f0)

    typical_slope = SR / (Q * HOP_LEN)
    gamma = math.tan(ts * math.pi / 2.0) * typical_slope / 4.0

    use_exp = abs(gamma) >= TAYLOR_CUT

    # padded grain count per core -> multiple of BATCH
    gpc_pad = ((GPC + BATCH - 1) // BATCH) * BATCH   # 513
    n_batches = gpc_pad // BATCH

    import ml_dtypes

    def bsplit(x, n):
        """Split f64 array into n bf16 parts summing to ~x (24 bits for n=3)."""
        parts = []
        rem = np.array(x, dtype=np.float64, copy=True)
        for _ in range(n):
            h = rem.astype(ml_dtypes.bfloat16)
            parts.append(h)
            rem = rem - h.astype(np.float64)
        return parts

    pvec = np.arange(128, dtype=np.float64)
    # base lhsT rows [KB, 128] in bf16; theta matmul runs at bf16 rate.
    # exp branch rows:    [Eh,Eh,Eh,Em,Em,El, 1, 1, 1]
    #   paired rhs rows:  [Rh,Rm,Rl,Rh,Rm,Rh, bh,bm,bl]
    # taylor branch rows: [1,1,1, p,p,p, p2h,p2h,p2l, p3,p3, p4]
    #   paired rhs rows:  [c0h,c0m,c0l, c1h,c1m,c1l, c2h,c2l,c2h, c3h,c3l, c4h]
    KB = 9 if use_exp else 12
    lhs_base = np.zeros((KB, 128), dtype=np.float64)
    if use_exp:
        E = np.exp(gamma * pvec / SR)
        Eh, Em, El = bsplit(E, 3)
        for i, v in enumerate([Eh, Eh, Eh, Em, Em, El]):
            lhs_base[i] = np.asarray(v, dtype=np.float64)
        lhs_base[6] = lhs_base[7] = lhs_base[8] = 1.0
    else:
        lhs_base[0] = lhs_base[1] = lhs_base[2] = 1.0
        lhs_base[3] = lhs_base[4] = lhs_base[5] = pvec
        p2h, p2l = bsplit(pvec ** 2, 2)
        lhs_base[6] = lhs_base[7] = np.asarray(p2h, np.float64)
        lhs_base[8] = np.asarray(p2l, np.float64)
        p3h = bsplit(pvec ** 3, 1)[0]
        lhs_base[9] = lhs_base[10] = np.asarray(p3h, np.float64)
        lhs_base[11] = np.asarray(bsplit(pvec ** 4, 1)[0], np.float64)
    lhs_base_bf = lhs_base.astype(ml_dtypes.bfloat16)
    lhs_base_f64 = lhs_base_bf.astype(np.float64)

    cvec = np.arange(GCOLS, dtype=np.float64)
    ncols = gpc_pad * GCOLS

    fact = [1, 1, 2, 6, 24, 120]

    cores = []
    seg_S = 8  # shared across cores (program structure is per-core anyway)
    for c in range(N_CORES):
        gsel = np.arange(c * GPC, (c + 1) * GPC)
        q = on[gsel] // 128
        order = np.argsort(q, kind="stable")
        gsel = gsel[order]
        q = q[order]
        r = on[gsel] % 128

        # strip assignment (greedy, span <= STRIP_COLS, no coverage gaps)
        strips = []  # list of [base, first_idx, last_idx, covered_end]
        base = None
        for k in range(GPC):
            qk = int(q[k])
            if (base is None or qk + GCOLS > base + STRIP_COLS
                    or qk > strips[-1][3]):
                base = qk
                strips.append([base, k, k, qk + GCOLS])
            else:
                strips[-1][2] = k
                strips[-1][3] = max(strips[-1][3], qk + GCOLS)
        # dummies join the last strip
        n_dummy = gpc_pad - GPC
        q_dummy = strips[-1][0]
        strips[-1][2] = gpc_pad - 1

        f0c = f0[gsel]
        Ac = A[gsel]

        # ideal theta model at p=0 (for the per-column base), [GPC, 129]
        # beta[g, c] = (128*c - r_g)/SR - D/2   (t at p=0)
        beta = (128.0 * cvec[None, :] - r[:, None]) / SR - GRAIN_DUR_S / 2.0
        if use_exp:
            a_g = f0c / gamma
            R_ideal = (a_g[:, None]) * np.exp(gamma * beta)
            const_ideal = np.broadcast_to(-a_g[:, None], beta.shape)
            theta0 = R_ideal + const_ideal  # theta at p=0
        else:
            coeff = np.zeros((5, GPC, GCOLS), dtype=np.float64)  # j = 0..4
            for k in range(1, 6):
                gk = gamma ** (k - 1) / fact[k]
                for j in range(0, min(k, 4) + 1):
                    binom = math.comb(k, j)
                    coeff[j] += gk * binom * beta ** (k - j) * SR ** (-j)
            coeff *= f0c[None, :, None]
            theta0 = coeff[0]

        base_c = np.round(theta0)  # folded into the const row -> |theta'| small

        # build bf16-split rhs base rows [KB, ncols]
        rhs64 = np.zeros((KB, ncols), dtype=np.float64)

        def put(row, arr):
            rhs64[row, : GPC * GCOLS] = np.asarray(arr, np.float64).reshape(-1)

        if use_exp:
            Rh, Rm, Rl = bsplit(R_ideal, 3)
            bh, bm, bl = bsplit(const_ideal - base_c, 3)
            for i, v in enumerate([Rh, Rm, Rl, Rh, Rm, Rh, bh, bm, bl]):
                put(i, v)
        else:
            c0h, c0m, c0l = bsplit(coeff[0] - base_c, 3)
            c1h, c1m, c1l = bsplit(coeff[1], 3)
            c2h, c2l = bsplit(coeff[2], 2)
            c3h, c3l = bsplit(coeff[3], 2)
            c4h = bsplit(coeff[4], 1)[0]
            for i, v in enumerate([c0h, c0m, c0l, c1h, c1m, c1l,
                                   c2h, c2l, c2h, c3h, c3l, c4h]):
                put(i, v)
        rhs_base = rhs64.astype(ml_dtypes.bfloat16)

        # device-model theta (f64 sim of the bf16 matmul), [128, ncols]
        theta = lhs_base_f64.T @ rhs_base.astype(np.float64)

        # segment hints: S partition segments, integer hint per (segment, col)
        S = seg_S
        while True:
            t3 = theta.reshape(S, 128 // S, ncols)
            mid = 0.5 * (t3.max(axis=1) + t3.min(axis=1))
            rseg = np.round(mid)  # [S, ncols] integers
            resid = np.abs(t3 - rseg[:, None, :]).max()
            if resid <= SEG_MARGIN or S >= 64:
                break
            S *= 2
        seg_S = max(seg_S, S)

        cores.append(
            dict(
                rhs_base=rhs_base,
                theta=theta,
                r=r,
                q=np.concatenate([q, np.full(n_dummy, q_dummy, dtype=np.int64)]),
                strips=strips,
                Ac=Ac,
            )
        )

    # second pass: uniform S across cores; build final lhs/rhs (+wa) arrays
    S = seg_S
    KR = KB + S
    seg = 128 // S
    lhs = np.zeros((KR, 128), dtype=np.float64)
    lhs[:KB] = lhs_base_f64
    for k in range(S):
        lhs[KB + k, k * seg:(k + 1) * seg] = 1.0
    lhs_bf = lhs.astype(ml_dtypes.bfloat16)

    resid_max = 0.0
    fallback = False
    pvec_i = np.arange(128)
    for c in range(N_CORES):
        info = cores[c]
        theta = info.pop("theta")
        t3 = theta.reshape(S, seg, ncols)
        mid = 0.5 * (t3.max(axis=1) + t3.min(axis=1))
        rseg = np.round(mid)  # [S, ncols]
        resid = np.abs(t3 - rseg[:, None, :]).max()
        resid_max = max(resid_max, resid)
        assert np.abs(rseg).max() < 250, "segment hint exceeds bf16-exact range"
        rhs = np.zeros((KR, ncols), dtype=np.float64)
        rhs[:KB] = info.pop("rhs_base").astype(np.float64)
        rhs[KB:] = -rseg
        info["rhs"] = rhs.astype(ml_dtypes.bfloat16)

        # WA table: A_g * W(i), i = 128*c + p - r_g.  The sample-range mask is
        # dropped: outside [0, GRAIN_N) the Hann window value is ~(pi*i/N)^2
        # <= 6e-4, negligible vs the 2e-2 error budget.
        r = info.pop("r")
        Ac = info.pop("Ac")
        i_idx = (128 * cvec[None, None, :] + pvec_i[None, :, None]
                 - r[:, None, None])  # [GPC, 128, 129]
        W = np.sin(np.pi * i_idx / GRAIN_N) ** 2
        WA = (W * Ac[:, None, None]).transpose(1, 0, 2).reshape(128, GPC * GCOLS)
        wa_full = np.zeros((128, ncols), dtype=np.float64)
        wa_full[:, : GPC * GCOLS] = WA
        info["wa"] = wa_full.astype(ml_dtypes.bfloat16)

    if resid_max > SEG_MARGIN:
        fallback = True  # extreme chirp slope: per-element hints needed

    meta = dict(
        lhs=lhs_bf,
        gpc_pad=gpc_pad,
        n_batches=n_batches,
        use_exp=use_exp,
        gamma=gamma,
        ncols=ncols,
        KR=KR,
        fallback=fallback,
        resid=resid_max,
    )
    return cores, meta


def _build_program(cores, meta, single_core=False):
    import concourse.bacc as bacc
    import concourse.bass as bass
    import concourse.tile as tile
    import concourse.mybir as mybir
    from concourse import bass_utils  # noqa: F401

    ncols = meta["ncols"]
    n_batches = meta["n_batches"]
    KR = meta["KR"]

    nc = bacc.Bacc("TRN2", target_bir_lowering=False, debug=False,
                   num_devices=1 if single_core else N_CORES)
    f32 = mybir.dt.float32
    bf16 = mybir.dt.bfloat16

    d_lhs = nc.dram_tensor("lhs", [KR, 128], bf16, kind="ExternalInput").ap()
    d_rhs = nc.dram_tensor("rhs", [KR, ncols], bf16, kind="ExternalInput").ap()
    d_wa = nc.dram_tensor("wa", [128, ncols], bf16, kind="ExternalInput").ap()
    d_iden = nc.dram_tensor("iden", [128, 128], bf16, kind="ExternalInput").ap()
    d_out = nc.dram_tensor("out", [65536], f32, kind="ExternalOutput").ap()

    AF = mybir.ActivationFunctionType
    ALU = mybir.AluOpType
    TWO_PI = float(2.0 * np.pi)

    with tile.TileContext(nc) as tc, ExitStack() as octx:
        outer = octx.enter_context(tc.tile_pool(name="outer", bufs=1))
        acc = outer.tile([128, ACC_PAD_COLS], f32)
        nc.gpsimd.memset(acc[:], 0.0)
        iden = outer.tile([128, 128], bf16)
        nc.sync.dma_start(iden[:], d_iden[:])
        lhs_t = outer.tile([KR, 128], bf16)
        nc.sync.dma_start(lhs_t[:], d_lhs[:])

        if not single_core:
            dram = octx.enter_context(
                tc.tile_pool(name="dram", bufs=1, space="DRAM"))
            b_in = dram.tile([128, ACC_COLS], f32)

        def emit_core_body(core):
            """Returns nothing; flushes final acc columns eagerly into the
            reduce input (SPMD) or the output (single-core estimate)."""
            info = cores[core]
            q = info["q"]
            strips = info["strips"]

            if single_core:
                flush_dst = d_full
            else:
                flush_dst = b_in[:]

            with ExitStack() as ctx:
                rhsp = ctx.enter_context(
                    tc.tile_pool(name=f"rhs{core}", bufs=1))
                wap = ctx.enter_context(tc.tile_pool(name=f"wap{core}", bufs=8))
                sp = ctx.enter_context(tc.tile_pool(name=f"sp{core}", bufs=3))
                vp = ctx.enter_context(tc.tile_pool(name=f"vp{core}", bufs=3))
                thp = ctx.enter_context(
                    tc.tile_pool(name=f"th{core}", bufs=2, space="PSUM"))
                stp = ctx.enter_context(
                    tc.tile_pool(name=f"st{core}", bufs=2, space="PSUM"))

                # strip state machine across batches
                strip_iter = iter(strips)
                cur = next(strip_iter)
                cur_tile = None
                flushed = 0  # acc cols already written out
                NB = BATCH * GCOLS  # 1161

                def flush_to(limit):
                    """DMA-out final acc cols [flushed, limit).  Issued from
                    the otherwise-idle Pool queue: a flush's sem-wait on
                    pending strip evacs must not delay the SP-issued input
                    DMAs."""
                    nonlocal flushed
                    lim = min(limit, ACC_COLS)
                    if lim - flushed >= FLUSH_MIN:
                        nc.gpsimd.dma_start(
                            flush_dst[:, flushed:lim], acc[:, flushed:lim])
                        flushed = lim

                def emit_scatter(g0, t_v):
                    nonlocal cur, cur_tile
                    for j in range(BATCH):
                        g = g0 + j
                        # open new strip?
                        if g > cur[2]:
                            # evacuate finished strip (covered span)
                            w = cur[3] - cur[0]
                            nc.vector.tensor_add(
                                acc[:, cur[0]:cur[0] + w],
                                cur_tile[:, :w],
                                acc[:, cur[0]:cur[0] + w],
                            )
                            cur = next(strip_iter)
                            cur_tile = None
                            flush_to(cur[0])
                        first = cur_tile is None
                        if first:
                            cur_tile = stp.tile(
                                [128, STRIP_COLS], f32, tag="strip")
                        off = int(q[g]) - cur[0]
                        last = g == cur[2]
                        nc.tensor.matmul(
                            cur_tile[:, off:off + GCOLS],
                            iden[:],
                            t_v[:, j * GCOLS:(j + 1) * GCOLS],
                            start=first, stop=last,
                        )

                # software pipeline: scatter runs two batches behind the
                # theta->sin->window chain so PE's in-order queue never
                # blocks upcoming theta matmuls on this batch's v
                # (steady state: theta(b+2) executes during ACT(b+1) while
                # scatter(b) waits for v(b)).
                from collections import deque
                pending = deque()  # (g0, t_v) awaiting scatter
                SCATTER_LAG = 2

                # All input DMAs are issued upfront from SP, in an order
                # that puts the data on the (serialized) DMA engines exactly
                # as the pipeline consumes it: [rhs0, wa0, rhs1, wa1, ...]
                # for the first DMA_PRE chunks, then the remaining rhs
                # slices (rhs_all is one resident SBUF tile -- disjoint
                # slice writes, no pool-slot waits), then the remaining wa
                # chunks, which self-pace on their pool-slot waits.
                n_chunks = (n_batches + DMA_B - 1) // DMA_B
                rhs_all = rhsp.tile([KR, ncols], bf16, tag="rhs")
                wa_tiles = []

                def emit_rhs_dma(k):
                    col0 = k * DMA_B * NB
                    W2 = min(DMA_B, n_batches - k * DMA_B) * NB
                    nc.sync.dma_start(
                        rhs_all[:, col0:col0 + W2], d_rhs[:, col0:col0 + W2])

                def emit_wa_dma(k):
                    col0 = k * DMA_B * NB
                    W2 = min(DMA_B, n_batches - k * DMA_B) * NB
                    t = wap.tile([128, DMA_B * NB], bf16, tag="wa")
                    nc.sync.dma_start(t[:, :W2], d_wa[:, col0:col0 + W2])
                    wa_tiles.append(t)

                for k in range(min(DMA_PRE, n_chunks)):
                    emit_rhs_dma(k)
                    emit_wa_dma(k)
                for k in range(DMA_PRE, n_chunks):
                    emit_rhs_dma(k)
                for k in range(DMA_PRE, n_chunks):
                    emit_wa_dma(k)

                for b2 in range(0, n_batches, DMA_B):
                    nb2 = min(DMA_B, n_batches - b2)
                    t_wa2 = wa_tiles[b2 // DMA_B]

                    for b in range(b2, b2 + nb2):
                        part = (b - b2) * NB
                        g0 = b * BATCH
                        t_rhs = rhs_all[:, b * NB:(b + 1) * NB]
                        t_wa = t_wa2[:, part:part + NB]

                        th = thp.tile([128, 3 * 512], f32, tag="th")
                        # priority-bias the theta matmuls ahead of the
                        # previous batch's scatter matmuls so PE's in-order
                        # dispatch never blocks them behind a v-wait.
                        with tc.high_priority(offset=16):
                            for m in range(3):
                                sl = slice(
                                    m * GPB * GCOLS, (m + 1) * GPB * GCOLS)
                                nc.tensor.matmul(
                                    th[:, m * 512: m * 512 + GPB * GCOLS],
                                    lhs_t[:],
                                    t_rhs[:, sl],
                                    start=True, stop=True,
                                )
                        th3 = th[:].rearrange(
                            "p (b x) -> p b x", b=3)[:, :, :GPB * GCOLS]
                        t_s = sp.tile([128, NB], bf16, tag="s")
                        s3 = t_s[:].rearrange("p (b x) -> p b x", b=3)
                        nc.scalar.activation(s3, th3, AF.Sin, scale=TWO_PI)
                        t_v = vp.tile([128, NB], bf16, tag="v")
                        nc.vector.tensor_mul(t_v[:], t_s[:], t_wa[:])

                        pending.append((g0, t_v))
                        if len(pending) > SCATTER_LAG:
                            emit_scatter(*pending.popleft())
                while pending:
                    emit_scatter(*pending.popleft())
                # final strip
                w = cur[3] - cur[0]
                nc.vector.tensor_add(
                    acc[:, cur[0]:cur[0] + w],
                    cur_tile[:, :w],
                    acc[:, cur[0]:cur[0] + w],
                )
                # flush the remainder
                if flushed < ACC_COLS:
                    nc.gpsimd.dma_start(
                        flush_dst[:, flushed:ACC_COLS],
                        acc[:, flushed:ACC_COLS])
                del wa_tiles[:]

        if single_core:
            d_full = nc.dram_tensor(
                "full", [128, ACC_COLS], f32, kind="ExternalOutput").ap()
            emit_core_body(0)
        else:
            pid = nc.partition_id()
            for core in range(N_CORES):
                with tc.If(pid == core):
                    emit_core_body(core)

            # ---- shared epilog: reduce, normalize, output ----
            b_rs = dram.tile([16, ACC_COLS], f32)
            nc.gpsimd.collective_compute(
                "ReduceScatter",
                mybir.AluOpType.add,
                replica_groups=[list(range(N_CORES))],
                ins=[b_in[:].opt()],
                outs=[b_rs[:].opt()],
            )
            red = outer.tile([128, 512], f32)
            nc.sync.dma_start(
                red[:],
                b_rs[:].rearrange("a b -> (a b)").rearrange(
                    "(p c) -> p c", p=128))

            # sum of squares of the local chunk
            scr = outer.tile([128, 512], f32)
            sqcol = outer.tile([128, 1], f32)
            nc.scalar.activation(scr[:], red[:], AF.Square, accum_out=sqcol[:])
            ones = outer.tile([128, 128], f32)
            nc.vector.memset(ones[:], 1.0)
            psq = octx.enter_context(tc.tile_pool(name="psq", bufs=1, space="PSUM"))
            ps_s = psq.tile([1, 128], f32)
            nc.tensor.matmul(ps_s[:], sqcol[:], ones[:], start=True, stop=True)
            ssq = outer.tile([1, 128], f32)
            nc.vector.tensor_copy(ssq[:], ps_s[:])

            b_s1 = dram.tile([1, 128], f32)
            b_s2 = dram.tile([1, 128], f32)
            nc.sync.dma_start(b_s1[:], ssq[:])
            nc.gpsimd.collective_compute(
                "AllReduce",
                mybir.AluOpType.add,
                replica_groups=[list(range(N_CORES))],
                ins=[b_s1[:].opt()],
                outs=[b_s2[:].opt()],
            )
            gsq = outer.tile([1, 1], f32)
            nc.sync.dma_start(gsq[:], b_s2[:, 0:1])

            # rscale = rsqrt(gsq) with one Newton refinement
            nrm = outer.tile([1, 1], f32)
            nc.scalar.activation(nrm[:], gsq[:], AF.Sqrt)
            z0 = outer.tile([1, 1], f32)
            nc.vector.reciprocal(z0[:], nrm[:])
            z2 = outer.tile([1, 1], f32)
            nc.vector.tensor_mul(z2[:], z0[:], z0[:])
            t2 = outer.tile([1, 1], f32)
            nc.vector.tensor_mul(t2[:], z2[:], gsq[:])
            t3 = outer.tile([1, 1], f32)
            nc.vector.tensor_scalar(t3[:], t2[:], -0.5, 1.5, ALU.mult, ALU.add)
            z1 = outer.tile([1, 1], f32)
            nc.vector.tensor_mul(z1[:], z0[:], t3[:])

            # broadcast to 128 partitions via DRAM bounce
            b_z = dram.tile([1, 1], f32)
            nc.sync.dma_start(b_z[:], z1[:])
            zb = outer.tile([128, 1], f32)
            bz_ap = b_z[:]
            bcast = bass.AP(tensor=bz_ap.tensor, offset=bz_ap.offset,
                            ap=[[0, 128], [1, 1]])
            nc.sync.dma_start(zb[:], bcast)

            outt = outer.tile([128, 512], f32)
            nc.vector.tensor_scalar(outt[:], red[:], zb[:], None, ALU.mult)
            nc.sync.dma_start(
                d_out.rearrange("(p c) -> p c", p=128), outt[:])

    nc.compile()
    return nc


def estimate_hw_time_ns(theta_density, theta_slope, f0_freqs_hz, onsets):
    """Cost-model (TimelineSim) estimate of one core's execution, ns.

    Single-core variant: core 0's synthesis+scatter+evac plus the 2MB
    accumulator DMA-out (standing in for the ReduceScatter contribution).
    """
    from concourse.timeline_sim import TimelineSim

    cores, meta = _host_prep(theta_density, theta_slope, f0_freqs_hz, onsets)
    nc = _build_program(cores, meta, single_core=True)
    ts = TimelineSim(nc)
    ts.simulate()
    return float(ts.time)


def kernel(theta_density, theta_slope, f0_freqs_hz, onsets):
    import ml_dtypes
    from concourse import bass_utils

    cores, meta = _host_prep(theta_density, theta_slope, f0_freqs_hz, onsets)
    nc = _build_program(cores, meta)

    iden = np.eye(128, dtype=np.float32).astype(ml_dtypes.bfloat16)
    in_maps = []
    for c in range(N_CORES):
        info = cores[c]
        in_maps.append(
            dict(
                lhs=meta["lhs"],
                rhs=info["rhs"],
                wa=info["wa"],
                iden=iden,
            )
        )
    res = bass_utils.run_bass_kernel_spmd(
        nc, in_maps, core_ids=list(range(N_CORES)))

    X = np.zeros((ACC_COLS, 128), dtype=np.float32)
    for c in range(N_CORES):
        chunk = res.results[c]["out"].reshape(16, ACC_COLS)
        X[:, 16 * c:16 * (c + 1)] = chunk.T
    return X.reshape(-1).astype(np.float32)


if __name__ == "__main__":
    rng = np.random.default_rng(0)
    out = kernel(
        np.float32(0.5), np.float32(0.3),
        np.exp(rng.uniform(np.log(F0_MIN), np.log(F0_MAX), N_GRAINS)).astype(np.float32),
        rng.integers(0, N_SAMPLES - GRAIN_N, N_GRAINS).astype(np.int32),
    )
    print(out.shape, out[:8], np.linalg.norm(out))



# revision 25
# speedup vs baseline: 1.2028x; 1.2028x over previous
"""ChirpTextureSynth Trainium2 kernel.

Synthesizes 4096 windowed chirp grains (16384 samples each), scatter-adds
them at per-grain onsets into a 524288-sample signal, L2-normalizes.

Strategy (8 NeuronCores, data-parallel over grains, 512 grains/core):
 - Output accumulator layout: sample s -> (partition p = s % 128, col = s // 128).
   A grain at onset o occupies cols [o//128, o//128 + 129) on all partitions
   (onsets never wrap: o < N_SAMPLES - GRAIN_N).
 - Sine argument in CYCLES: theta(p,c) = f0*phase(t), t = i/SR - D/2,
   i = 128*c + p - (o % 128). theta is low-rank separable in (p, c):
   exp branch  : theta = a*E(p)*F(c) + b,  E(p)=exp(g*p/SR)
   taylor branch (|g| < 0.7): theta = sum_j coeff_j(c) * p^j, j=0..4
 - Range reduction is folded INTO the theta matmul: S piecewise-constant
   partition-segment hint rows (indicator lhsT rows x integer bf16 rhs rows)
   subtract round(theta) per segment, leaving |u| <= ~0.58.  The ACT Sin
   spline (scale=2pi) is accurate to ~3e-4 out to |u|=0.58, so ONE K<=128
   matmul per 3 grains produces ready-to-sin u tiles in PSUM (f32).
   Fallback for extreme chirp slopes (segment residual too big): per-element
   int8 hint plane applied via identity matmul / DVE, as before.
 - ACT Sin(scale=2pi) evaluates sin(2*pi*u) -> bf16 SBUF.
 - Window*amp (bf16 host table WA) applied on DVE: v = s * WA (2x bf16).
 - Scatter: per-grain matmul with identity weights accumulates v into a
   PSUM "strip" bank (512 cols); strips follow onset-sorted grains; DVE
   evacuates each strip into the SBUF accumulator.  Columns left of the
   next strip's base are final and are streamed out to DRAM eagerly.
 - Per-core instruction streams differ (grain offsets are immediates), so the
   program has 8 tc.If(partition_id == c) branches; inputs differ per core.
 - Reduction: ReduceScatter (128x4096 f32) + scalar AllReduce of sum-of-
   squares; each core normalizes and outputs its 1/8 chunk; host reassembles.
"""

import math
from contextlib import ExitStack

import numpy as np

SR = 44100.0
N_SAMPLES = 524288
N_GRAINS = 4096
GRAIN_N = 16384
F0_MIN = 32.7
F0_MAX = 523.25
Q = 12
HOP_LEN = 256
GRAIN_DUR_S = GRAIN_N / SR
N_CORES = 8
GPC = N_GRAINS // N_CORES  # grains per core (512)

ACC_COLS = N_SAMPLES // 128        # 4096
ACC_PAD_COLS = ACC_COLS + 384      # strip overhang room
GCOLS = 117                        # cols per grain tile; (129-GCOLS)/2 cols
                                   # trimmed per edge where the Hann window is
                                   # tiny (rel-L2 cost ~10*f^2.5, f=trim frac)
BATCH = 12                         # grains per compute batch (3 psum banks)
GPB = 4                            # grains per theta-matmul (508 cols <= 512)
TAYLOR_CUT = 0.7                   # |gamma| below which the poly branch is used
STRIP_COLS = 512
SEG_MARGIN = 0.65                  # max |u| the Sin spline tolerates (~5e-3)
DMA_B = 2                          # batches per input DMA chunk
DMA_PRE = 5                        # chunks interleaved rhs/wa at the start
FLUSH_MIN = 384                    # min final cols before an output flush
SCATTER_LAG = 4                    # batches the scatter trails the sin chain


def _host_prep(theta_density, theta_slope, f0_freqs_hz, onsets):
    """All host-side precompute. Returns per-core input arrays + metadata."""
    td = float(np.float32(theta_density))
    ts = float(np.float32(theta_slope))
    f0 = np.asarray(f0_freqs_hz, dtype=np.float64)
    on = np.asarray(onsets, dtype=np.int64)

    # per-grain amplitudes (matches reference, f64 is fine vs f32 ref)
    gi = np.arange(N_GRAINS, dtype=np.float64)
    offset = 0.25 * td + 0.75 * td * td
    sig_op = (1.0 - td) * N_GRAINS * (gi / N_GRAINS - offset)
    amps = 0.5 * (1.0 - np.tanh(sig_op))  # = 1 - sigmoid(2*sig_op), stable
    amps = amps / amps.max()
    A = amps / np.sqrt(f0)

    typical_slope = SR / (Q * HOP_LEN)
    gamma = math.tan(ts * math.pi / 2.0) * typical_slope / 4.0

    use_exp = abs(gamma) >= TAYLOR_CUT

    # padded grain count per core -> multiple of BATCH
    gpc_pad = ((GPC + BATCH - 1) // BATCH) * BATCH   # 513
    n_batches = gpc_pad // BATCH

    import ml_dtypes

    def bsplit(x, n):
        """Split f64 array into n bf16 parts summing to ~x (24 bits for n=3)."""
        parts = []
        rem = np.array(x, dtype=np.float64, copy=True)
        for _ in range(n):
            h = rem.astype(ml_dtypes.bfloat16)
            parts.append(h)
            rem = rem - h.astype(np.float64)
        return parts

    pvec = np.arange(128, dtype=np.float64)
    # base lhsT rows [KB, 128] in bf16; theta matmul runs at bf16 rate.
    # exp branch rows:    [Eh,Eh,Eh,Em,Em,El, 1, 1, 1]
    #   paired rhs rows:  [Rh,Rm,Rl,Rh,Rm,Rh, bh,bm,bl]
    # taylor branch rows: [1,1,1, p,p,p, p2h,p2h,p2l, p3,p3, p4]
    #   paired rhs rows:  [c0h,c0m,c0l, c1h,c1m,c1l, c2h,c2l,c2h, c3h,c3l, c4h]
    KB = 9 if use_exp else 12
    lhs_base = np.zeros((KB, 128), dtype=np.float64)
    if use_exp:
        E = np.exp(gamma * pvec / SR)
        Eh, Em, El = bsplit(E, 3)
        for i, v in enumerate([Eh, Eh, Eh, Em, Em, El]):
            lhs_base[i] = np.asarray(v, dtype=np.float64)
        lhs_base[6] = lhs_base[7] = lhs_base[8] = 1.0
    else:
        lhs_base[0] = lhs_base[1] = lhs_base[2] = 1.0
        lhs_base[3] = lhs_base[4] = lhs_base[5] = pvec
        p2h, p2l = bsplit(pvec ** 2, 2)
        lhs_base[6] = lhs_base[7] = np.asarray(p2h, np.float64)
        lhs_base[8] = np.asarray(p2l, np.float64)
        p3h = bsplit(pvec ** 3, 1)[0]
        lhs_base[9] = lhs_base[10] = np.asarray(p3h, np.float64)
        lhs_base[11] = np.asarray(bsplit(pvec ** 4, 1)[0], np.float64)
    lhs_base_bf = lhs_base.astype(ml_dtypes.bfloat16)
    lhs_base_f64 = lhs_base_bf.astype(np.float64)

    # tile col c covers samples i = 128*(c+TRIM_L) + p - r (grain-local),
    # i.e. global cols q+c with q = onset//128 + TRIM_L: the (129-GCOLS)
    # dropped edge cols carry only the Hann window's faint tails.
    TRIM_L = (129 - GCOLS) // 2
    cvec = np.arange(GCOLS, dtype=np.float64) + TRIM_L
    ncols = gpc_pad * GCOLS

    fact = [1, 1, 2, 6, 24, 120]

    cores = []
    seg_S = 8  # shared across cores (program structure is per-core anyway)
    for c in range(N_CORES):
        gsel = np.arange(c * GPC, (c + 1) * GPC)
        q = on[gsel] // 128 + TRIM_L
        order = np.argsort(q, kind="stable")
        gsel = gsel[order]
        q = q[order]
        r = on[gsel] % 128

        # strip assignment (greedy, span <= STRIP_COLS, no coverage gaps)
        strips = []  # list of [base, first_idx, last_idx, covered_end]
        base = None
        for k in range(GPC):
            qk = int(q[k])
            if (base is None or qk + GCOLS > base + STRIP_COLS
                    or qk > strips[-1][3]):
                base = qk
                strips.append([base, k, k, qk + GCOLS])
            else:
                strips[-1][2] = k
                strips[-1][3] = max(strips[-1][3], qk + GCOLS)
        # dummies join the last strip
        n_dummy = gpc_pad - GPC
        q_dummy = strips[-1][0]
        strips[-1][2] = gpc_pad - 1

        f0c = f0[gsel]
        Ac = A[gsel]

        # ideal theta model at p=0 (for the per-column base), [GPC, 129]
        # beta[g, c] = (128*c - r_g)/SR - D/2   (t at p=0)
        beta = (128.0 * cvec[None, :] - r[:, None]) / SR - GRAIN_DUR_S / 2.0
        if use_exp:
            a_g = f0c / gamma
            R_ideal = (a_g[:, None]) * np.exp(gamma * beta)
            const_ideal = np.broadcast_to(-a_g[:, None], beta.shape)
            theta0 = R_ideal + const_ideal  # theta at p=0
        else:
            coeff = np.zeros((5, GPC, GCOLS), dtype=np.float64)  # j = 0..4
            for k in range(1, 6):
                gk = gamma ** (k - 1) / fact[k]
                for j in range(0, min(k, 4) + 1):
                    binom = math.comb(k, j)
                    coeff[j] += gk * binom * beta ** (k - j) * SR ** (-j)
            coeff *= f0c[None, :, None]
            theta0 = coeff[0]

        base_c = np.round(theta0)  # folded into the const row -> |theta'| small

        # build bf16-split rhs base rows [KB, ncols]
        rhs64 = np.zeros((KB, ncols), dtype=np.float64)

        def put(row, arr):
            rhs64[row, : GPC * GCOLS] = np.asarray(arr, np.float64).reshape(-1)

        if use_exp:
            Rh, Rm, Rl = bsplit(R_ideal, 3)
            bh, bm, bl = bsplit(const_ideal - base_c, 3)
            for i, v in enumerate([Rh, Rm, Rl, Rh, Rm, Rh, bh, bm, bl]):
                put(i, v)
        else:
            c0h, c0m, c0l = bsplit(coeff[0] - base_c, 3)
            c1h, c1m, c1l = bsplit(coeff[1], 3)
            c2h, c2l = bsplit(coeff[2], 2)
            c3h, c3l = bsplit(coeff[3], 2)
            c4h = bsplit(coeff[4], 1)[0]
            for i, v in enumerate([c0h, c0m, c0l, c1h, c1m, c1l,
                                   c2h, c2l, c2h, c3h, c3l, c4h]):
                put(i, v)
        rhs_base = rhs64.astype(ml_dtypes.bfloat16)

        # device-model theta (f64 sim of the bf16 matmul), [128, ncols]
        theta = lhs_base_f64.T @ rhs_base.astype(np.float64)

        # segment hints: S partition segments, integer hint per (segment, col)
        S = seg_S
        while True:
            t3 = theta.reshape(S, 128 // S, ncols)
            mid = 0.5 * (t3.max(axis=1) + t3.min(axis=1))
            rseg = np.round(mid)  # [S, ncols] integers
            resid = np.abs(t3 - rseg[:, None, :]).max()
            if resid <= SEG_MARGIN or S >= 64:
                break
            S *= 2
        seg_S = max(seg_S, S)

        cores.append(
            dict(
                rhs_base=rhs_base,
                theta=theta,
                r=r,
                q=np.concatenate([q, np.full(n_dummy, q_dummy, dtype=np.int64)]),
                strips=strips,
                Ac=Ac,
            )
        )

    # second pass: uniform S across cores; build final lhs/rhs (+wa) arrays
    S = seg_S
    KR = KB + S
    seg = 128 // S
    lhs = np.zeros((KR, 128), dtype=np.float64)
    lhs[:KB] = lhs_base_f64
    for k in range(S):
        lhs[KB + k, k * seg:(k + 1) * seg] = 1.0
    lhs_bf = lhs.astype(ml_dtypes.bfloat16)

    resid_max = 0.0
    fallback = False
    pvec_i = np.arange(128)
    for c in range(N_CORES):
        info = cores[c]
        theta = info.pop("theta")
        t3 = theta.reshape(S, seg, ncols)
        mid = 0.5 * (t3.max(axis=1) + t3.min(axis=1))
        rseg = np.round(mid)  # [S, ncols]
        resid = np.abs(t3 - rseg[:, None, :]).max()
        resid_max = max(resid_max, resid)
        assert np.abs(rseg).max() < 250, "segment hint exceeds bf16-exact range"
        rhs = np.zeros((KR, ncols), dtype=np.float64)
        rhs[:KB] = info.pop("rhs_base").astype(np.float64)
        rhs[KB:] = -rseg
        info["rhs"] = rhs.astype(ml_dtypes.bfloat16)

        # WA table: A_g * W(i), i = 128*c + p - r_g.  The sample-range mask is
        # dropped: outside [0, GRAIN_N) the Hann window value is ~(pi*i/N)^2
        # <= 6e-4, negligible vs the 2e-2 error budget.
        r = info.pop("r")
        Ac = info.pop("Ac")
        i_idx = (128 * cvec[None, None, :] + pvec_i[None, :, None]
                 - r[:, None, None])  # [GPC, 128, 129]
        W = np.sin(np.pi * i_idx / GRAIN_N) ** 2
        WA = (W * Ac[:, None, None]).transpose(1, 0, 2).reshape(128, GPC * GCOLS)
        wa_full = np.zeros((128, ncols), dtype=np.float64)
        wa_full[:, : GPC * GCOLS] = WA
        info["wa"] = wa_full.astype(ml_dtypes.bfloat16)

    if resid_max > SEG_MARGIN:
        fallback = True  # extreme chirp slope: per-element hints needed

    meta = dict(
        lhs=lhs_bf,
        gpc_pad=gpc_pad,
        n_batches=n_batches,
        use_exp=use_exp,
        gamma=gamma,
        ncols=ncols,
        KR=KR,
        fallback=fallback,
        resid=resid_max,
    )
    return cores, meta


def _build_program(cores, meta, single_core=False):
    import concourse.bacc as bacc
    import concourse.bass as bass
    import concourse.tile as tile
    import concourse.mybir as mybir
    from concourse import bass_utils  # noqa: F401

    ncols = meta["ncols"]
    n_batches = meta["n_batches"]
    KR = meta["KR"]

    nc = bacc.Bacc("TRN2", target_bir_lowering=False, debug=False,
                   num_devices=1 if single_core else N_CORES)
    f32 = mybir.dt.float32
    bf16 = mybir.dt.bfloat16

    d_lhs = nc.dram_tensor("lhs", [KR, 128], bf16, kind="ExternalInput").ap()
    d_rhs = nc.dram_tensor("rhs", [KR, ncols], bf16, kind="ExternalInput").ap()
    d_wa = nc.dram_tensor("wa", [128, ncols], bf16, kind="ExternalInput").ap()
    d_iden = nc.dram_tensor("iden", [128, 128], bf16, kind="ExternalInput").ap()
    d_out = nc.dram_tensor("out", [65536], f32, kind="ExternalOutput").ap()

    AF = mybir.ActivationFunctionType
    ALU = mybir.AluOpType
    TWO_PI = float(2.0 * np.pi)

    with tile.TileContext(nc) as tc, ExitStack() as octx:
        outer = octx.enter_context(tc.tile_pool(name="outer", bufs=1))
        acc = outer.tile([128, ACC_PAD_COLS], f32)
        # memset on DVE (idle at startup): on Pool it would sit ahead of the
        # rhs SWDGE descriptor generations and delay the first theta by ~4us
        nc.vector.memset(acc[:], 0.0)
        lhs_t = outer.tile([KR, 128], bf16)
        nc.sync.dma_start(lhs_t[:], d_lhs[:])  # tiny, ahead of rhs0 on SP
        iden = outer.tile([128, 128], bf16)
        # iden's DMA is issued inside the core body after the first input
        # chunk: it is first read only at the first scatter (~10us in) and
        # must not delay rhs chunk 0 on the DMA engines.

        if not single_core:
            dram = octx.enter_context(
                tc.tile_pool(name="dram", bufs=1, space="DRAM"))
            b_in = dram.tile([128, ACC_COLS], f32)

        def emit_core_body(core):
            """Returns nothing; flushes final acc columns eagerly into the
            reduce input (SPMD) or the output (single-core estimate)."""
            info = cores[core]
            q = info["q"]
            strips = info["strips"]

            if single_core:
                flush_dst = d_full
            else:
                flush_dst = b_in[:]

            with ExitStack() as ctx:
                rhsp = ctx.enter_context(
                    tc.tile_pool(name=f"rhs{core}", bufs=1))
                wap = ctx.enter_context(tc.tile_pool(name=f"wap{core}", bufs=5))
                sp = ctx.enter_context(tc.tile_pool(name=f"sp{core}", bufs=3))
                vp = ctx.enter_context(
                    tc.tile_pool(name=f"vp{core}", bufs=SCATTER_LAG + 2))
                thp = ctx.enter_context(
                    tc.tile_pool(name=f"th{core}", bufs=2, space="PSUM"))
                stp = ctx.enter_context(
                    tc.tile_pool(name=f"st{core}", bufs=2, space="PSUM"))

                # strip state machine across batches
                strip_iter = iter(strips)
                cur = next(strip_iter)
                cur_tile = None
                flushed = 0  # acc cols already written out
                NB = BATCH * GCOLS  # 1161

                def flush_to(limit, force=False):
                    """DMA-out final acc cols [flushed, limit).  Issued from
                    the otherwise-idle Pool queue: a flush's sem-wait on
                    pending strip evacs must not delay the SP-issued input
                    DMAs."""
                    nonlocal flushed
                    lim = min(limit, ACC_COLS)
                    if lim - flushed >= FLUSH_MIN or (force and lim > flushed):
                        nc.gpsimd.dma_start(
                            flush_dst[:, flushed:lim], acc[:, flushed:lim])
                        flushed = lim

                def emit_scatter(g0, t_v):
                    nonlocal cur, cur_tile
                    for j in range(BATCH):
                        g = g0 + j
                        # open new strip?
                        if g > cur[2]:
                            # evacuate finished strip (covered span)
                            w = cur[3] - cur[0]
                            nc.vector.tensor_add(
                                acc[:, cur[0]:cur[0] + w],
                                cur_tile[:, :w],
                                acc[:, cur[0]:cur[0] + w],
                            )
                            cur = next(strip_iter)
                            cur_tile = None
                            flush_to(cur[0])
                        first = cur_tile is None
                        if first:
                            cur_tile = stp.tile(
                                [128, STRIP_COLS], f32, tag="strip")
                        off = int(q[g]) - cur[0]
                        last = g == cur[2]
                        nc.tensor.matmul(
                            cur_tile[:, off:off + GCOLS],
                            iden[:],
                            t_v[:, j * GCOLS:(j + 1) * GCOLS],
                            start=first, stop=last,
                        )

                # software pipeline: scatter runs SCATTER_LAG batches behind
                # the theta->sin->window chain so PE's in-order queue never
                # blocks upcoming theta matmuls on a v that was only just
                # produced (the ACT->DVE->v chain is ~1.2us deep).
                from collections import deque
                pending = deque()  # (g0, t_v) awaiting scatter

                # Input DMA routing: all rhs chunks are issued upfront from
                # the Pool queue (SWDGE) so they are never queued behind a wa
                # pool-slot wait -- if they were, the Tile scheduler's own
                # pipeline sim would see theta matmuls blocked on rhs and lock
                # a scatter-before-theta PE order that stalls ACT at runtime.
                # wa chunks stream on SP, self-paced by their pool slots.
                n_chunks = (n_batches + DMA_B - 1) // DMA_B
                rhs_all = rhsp.tile([KR, ncols], bf16, tag="rhs")
                wa_tiles = []

                def emit_rhs_dma(k):
                    col0 = k * DMA_B * NB
                    W2 = min(DMA_B, n_batches - k * DMA_B) * NB
                    # chunk 0 from SP (fast HWDGE, ahead of wa0 on the DMA
                    # engines -- it gates the first theta); the rest from
                    # Pool/SWDGE so they never sit behind a wa slot wait
                    eng = nc.sync if k == 0 else nc.gpsimd
                    eng.dma_start(
                        rhs_all[:, col0:col0 + W2], d_rhs[:, col0:col0 + W2])

                def emit_wa_dma(k):
                    col0 = k * DMA_B * NB
                    W2 = min(DMA_B, n_batches - k * DMA_B) * NB
                    t = wap.tile([128, DMA_B * NB], bf16, tag="wa")
                    nc.sync.dma_start(t[:, :W2], d_wa[:, col0:col0 + W2])
                    wa_tiles.append(t)

                for k in range(n_chunks):
                    emit_rhs_dma(k)
                for k in range(n_chunks):
                    emit_wa_dma(k)
                    if k == 0:
                        nc.sync.dma_start(iden[:], d_iden[:])

                for b2 in range(0, n_batches, DMA_B):
                    nb2 = min(DMA_B, n_batches - b2)
                    t_wa2 = wa_tiles[b2 // DMA_B]

                    for b in range(b2, b2 + nb2):
                        part = (b - b2) * NB
                        g0 = b * BATCH
                        t_rhs = rhs_all[:, b * NB:(b + 1) * NB]
                        t_wa = t_wa2[:, part:part + NB]

                        th = thp.tile([128, 3 * 512], f32, tag="th")
                        # priority-bias the theta matmuls ahead of the
                        # previous batch's scatter matmuls so PE's in-order
                        # dispatch never blocks them behind a v-wait.
                        with tc.high_priority(offset=16):
                            for m in range(3):
                                sl = slice(
                                    m * GPB * GCOLS, (m + 1) * GPB * GCOLS)
                                nc.tensor.matmul(
                                    th[:, m * 512: m * 512 + GPB * GCOLS],
                                    lhs_t[:],
                                    t_rhs[:, sl],
                                    start=True, stop=True,
                                )
                        th3 = th[:].rearrange(
                            "p (b x) -> p b x", b=3)[:, :, :GPB * GCOLS]
                        t_s = sp.tile([128, NB], bf16, tag="s")
                        s3 = t_s[:].rearrange("p (b x) -> p b x", b=3)
                        nc.scalar.activation(s3, th3, AF.Sin, scale=TWO_PI)
                        t_v = vp.tile([128, NB], bf16, tag="v")
                        nc.vector.tensor_mul(t_v[:], t_s[:], t_wa[:])

                        pending.append((g0, t_v))
                        if len(pending) > SCATTER_LAG:
                            emit_scatter(*pending.popleft())
                while pending:
                    emit_scatter(*pending.popleft())
                # final strip
                w = cur[3] - cur[0]
                nc.vector.tensor_add(
                    acc[:, cur[0]:cur[0] + w],
                    cur_tile[:, :w],
                    acc[:, cur[0]:cur[0] + w],
                )
                # flush the remainder
                if flushed < ACC_COLS:
                    nc.gpsimd.dma_start(
                        flush_dst[:, flushed:ACC_COLS],
                        acc[:, flushed:ACC_COLS])
                del wa_tiles[:]

        if single_core:
            d_full = nc.dram_tensor(
                "full", [128, ACC_COLS], f32, kind="ExternalOutput").ap()
            emit_core_body(0)
        else:
            pid = nc.partition_id()
            for core in range(N_CORES):
                with tc.If(pid == core):
                    emit_core_body(core)

            # ---- shared epilog: reduce, normalize, output ----
            b_rs = dram.tile([16, ACC_COLS], f32)
            nc.gpsimd.collective_compute(
                "ReduceScatter",
                mybir.AluOpType.add,
                replica_groups=[list(range(N_CORES))],
                ins=[b_in[:].opt()],
                outs=[b_rs[:].opt()],
            )
            red = outer.tile([128, 512], f32)
            nc.sync.dma_start(
                red[:],
                b_rs[:].rearrange("a b -> (a b)").rearrange(
                    "(p c) -> p c", p=128))

            # sum of squares of the local chunk
            scr = outer.tile([128, 512], f32)
            sqcol = outer.tile([128, 1], f32)
            nc.scalar.activation(scr[:], red[:], AF.Square, accum_out=sqcol[:])
            ones = outer.tile([128, 128], f32)
            nc.vector.memset(ones[:], 1.0)
            psq = octx.enter_context(tc.tile_pool(name="psq", bufs=1, space="PSUM"))
            ps_s = psq.tile([1, 128], f32)
            nc.tensor.matmul(ps_s[:], sqcol[:], ones[:], start=True, stop=True)
            ssq = outer.tile([1, 128], f32)
            nc.vector.tensor_copy(ssq[:], ps_s[:])

            b_s1 = dram.tile([1, 128], f32)
            b_s2 = dram.tile([1, 128], f32)
            nc.sync.dma_start(b_s1[:], ssq[:])
            nc.gpsimd.collective_compute(
                "AllReduce",
                mybir.AluOpType.add,
                replica_groups=[list(range(N_CORES))],
                ins=[b_s1[:].opt()],
                outs=[b_s2[:].opt()],
            )
            gsq = outer.tile([1, 1], f32)
            nc.sync.dma_start(gsq[:], b_s2[:, 0:1])

            # rscale = rsqrt(gsq) with one Newton refinement
            nrm = outer.tile([1, 1], f32)
            nc.scalar.activation(nrm[:], gsq[:], AF.Sqrt)
            z0 = outer.tile([1, 1], f32)
            nc.vector.reciprocal(z0[:], nrm[:])
            z2 = outer.tile([1, 1], f32)
            nc.vector.tensor_mul(z2[:], z0[:], z0[:])
            t2 = outer.tile([1, 1], f32)
            nc.vector.tensor_mul(t2[:], z2[:], gsq[:])
            t3 = outer.tile([1, 1], f32)
            nc.vector.tensor_scalar(t3[:], t2[:], -0.5, 1.5, ALU.mult, ALU.add)
            z1 = outer.tile([1, 1], f32)
            nc.vector.tensor_mul(z1[:], z0[:], t3[:])

            # broadcast to 128 partitions via DRAM bounce
            b_z = dram.tile([1, 1], f32)
            nc.sync.dma_start(b_z[:], z1[:])
            zb = outer.tile([128, 1], f32)
            bz_ap = b_z[:]
            bcast = bass.AP(tensor=bz_ap.tensor, offset=bz_ap.offset,
                            ap=[[0, 128], [1, 1]])
            nc.sync.dma_start(zb[:], bcast)

            outt = outer.tile([128, 512], f32)
            nc.vector.tensor_scalar(outt[:], red[:], zb[:], None, ALU.mult)
            nc.sync.dma_start(
                d_out.rearrange("(p c) -> p c", p=128), outt[:])

    nc.compile()
    return nc


def estimate_hw_time_ns(theta_density, theta_slope, f0_freqs_hz, onsets):
    """Cost-model (TimelineSim) estimate of one core's execution, ns.

    Single-core variant: core 0's synthesis+scatter+evac plus the 2MB
    accumulator DMA-out (standing in for the ReduceScatter contribution).
    """
    from concourse.timeline_sim import TimelineSim

    cores, meta = _host_prep(theta_density, theta_slope, f0_freqs_hz, onsets)
    nc = _build_program(cores, meta, single_core=True)
    ts = TimelineSim(nc)
    ts.simulate()
    return float(ts.time)


def kernel(theta_density, theta_slope, f0_freqs_hz, onsets):
    import ml_dtypes
    from concourse import bass_utils

    cores, meta = _host_prep(theta_density, theta_slope, f0_freqs_hz, onsets)
    nc = _build_program(cores, meta)

    iden = np.eye(128, dtype=np.float32).astype(ml_dtypes.bfloat16)
    in_maps = []
    for c in range(N_CORES):
        info = cores[c]
        in_maps.append(
            dict(
                lhs=meta["lhs"],
                rhs=info["rhs"],
                wa=info["wa"],
                iden=iden,
            )
        )
    res = bass_utils.run_bass_kernel_spmd(
        nc, in_maps, core_ids=list(range(N_CORES)))

    X = np.zeros((ACC_COLS, 128), dtype=np.float32)
    for c in range(N_CORES):
        chunk = res.results[c]["out"].reshape(16, ACC_COLS)
        X[:, 16 * c:16 * (c + 1)] = chunk.T
    return X.reshape(-1).astype(np.float32)


if __name__ == "__main__":
    rng = np.random.default_rng(0)
    out = kernel(
        np.float32(0.5), np.float32(0.3),
        np.exp(rng.uniform(np.log(F0_MIN), np.log(F0_MAX), N_GRAINS)).astype(np.float32),
        rng.integers(0, N_SAMPLES - GRAIN_N, N_GRAINS).astype(np.int32),
    )
    print(out.shape, out[:8], np.linalg.norm(out))



# revision 32
# speedup vs baseline: 1.2184x; 1.0129x over previous
"""ChirpTextureSynth Trainium2 kernel.

Synthesizes 4096 windowed chirp grains (16384 samples each), scatter-adds
them at per-grain onsets into a 524288-sample signal, L2-normalizes.

Strategy (8 NeuronCores, data-parallel over grains, 512 grains/core):
 - Output accumulator layout: sample s -> (partition p = s % 128, col = s // 128).
   A grain at onset o occupies cols [o//128, o//128 + 129) on all partitions
   (onsets never wrap: o < N_SAMPLES - GRAIN_N).
 - Sine argument in CYCLES: theta(p,c) = f0*phase(t), t = i/SR - D/2,
   i = 128*c + p - (o % 128). theta is low-rank separable in (p, c):
   exp branch  : theta = a*E(p)*F(c) + b,  E(p)=exp(g*p/SR)
   taylor branch (|g| < 0.7): theta = sum_j coeff_j(c) * p^j, j=0..4
 - Range reduction is folded INTO the theta matmul: S piecewise-constant
   partition-segment hint rows (indicator lhsT rows x integer bf16 rhs rows)
   subtract round(theta) per segment, leaving |u| <= ~0.58.  The ACT Sin
   spline (scale=2pi) is accurate to ~3e-4 out to |u|=0.58, so ONE K<=128
   matmul per 3 grains produces ready-to-sin u tiles in PSUM (f32).
   Fallback for extreme chirp slopes (segment residual too big): per-element
   int8 hint plane applied via identity matmul / DVE, as before.
 - ACT Sin(scale=2pi) evaluates sin(2*pi*u) -> bf16 SBUF.
 - Window*amp (bf16 host table WA) applied on DVE: v = s * WA (2x bf16).
 - Scatter: per-grain matmul with identity weights accumulates v into a
   PSUM "strip" bank (512 cols); strips follow onset-sorted grains; DVE
   evacuates each strip into the SBUF accumulator.  Columns left of the
   next strip's base are final and are streamed out to DRAM eagerly.
 - Per-core instruction streams differ (grain offsets are immediates), so the
   program has 8 tc.If(partition_id == c) branches; inputs differ per core.
 - Reduction: ReduceScatter (128x4096 f32) + scalar AllReduce of sum-of-
   squares; each core normalizes and outputs its 1/8 chunk; host reassembles.
"""

import math
from contextlib import ExitStack

import numpy as np

SR = 44100.0
N_SAMPLES = 524288
N_GRAINS = 4096
GRAIN_N = 16384
F0_MIN = 32.7
F0_MAX = 523.25
Q = 12
HOP_LEN = 256
GRAIN_DUR_S = GRAIN_N / SR
N_CORES = 8
GPC = N_GRAINS // N_CORES  # grains per core (512)

ACC_COLS = N_SAMPLES // 128        # 4096
ACC_PAD_COLS = ACC_COLS + 384      # strip overhang room
GCOLS = 117                        # cols per grain tile; (129-GCOLS)/2 cols
                                   # trimmed per edge where the Hann window is
                                   # tiny (rel-L2 cost ~10*f^2.5, f=trim frac)
BATCH = 12                         # grains per compute batch (3 psum banks)
GPB = 4                            # grains per theta-matmul (508 cols <= 512)
TAYLOR_CUT = 0.7                   # |gamma| below which the poly branch is used
STRIP_COLS = 512
SEG_MARGIN = 0.65                  # max |u| the Sin spline tolerates (~5e-3)
DMA_B = 2                          # batches per input DMA chunk
DMA_PRE = 5                        # chunks interleaved rhs/wa at the start
FLUSH_MIN = 384                    # min final cols before an output flush
SCATTER_LAG = 4                    # batches the scatter trails the sin chain


def _host_prep(theta_density, theta_slope, f0_freqs_hz, onsets):
    """All host-side precompute. Returns per-core input arrays + metadata."""
    td = float(np.float32(theta_density))
    ts = float(np.float32(theta_slope))
    f0 = np.asarray(f0_freqs_hz, dtype=np.float64)
    on = np.asarray(onsets, dtype=np.int64)

    # per-grain amplitudes (matches reference, f64 is fine vs f32 ref)
    gi = np.arange(N_GRAINS, dtype=np.float64)
    offset = 0.25 * td + 0.75 * td * td
    sig_op = (1.0 - td) * N_GRAINS * (gi / N_GRAINS - offset)
    amps = 0.5 * (1.0 - np.tanh(sig_op))  # = 1 - sigmoid(2*sig_op), stable
    amps = amps / amps.max()
    A = amps / np.sqrt(f0)

    typical_slope = SR / (Q * HOP_LEN)
    gamma = math.tan(ts * math.pi / 2.0) * typical_slope / 4.0

    use_exp = abs(gamma) >= TAYLOR_CUT

    # padded grain count per core -> multiple of BATCH
    gpc_pad = ((GPC + BATCH - 1) // BATCH) * BATCH   # 513
    n_batches = gpc_pad // BATCH

    import ml_dtypes

    def bsplit(x, n):
        """Split f64 array into n bf16 parts summing to ~x (24 bits for n=3)."""
        parts = []
        rem = np.array(x, dtype=np.float64, copy=True)
        for _ in range(n):
            h = rem.astype(ml_dtypes.bfloat16)
            parts.append(h)
            rem = rem - h.astype(np.float64)
        return parts

    pvec = np.arange(128, dtype=np.float64)
    # base lhsT rows [KB, 128] in bf16; theta matmul runs at bf16 rate.
    # exp branch rows:    [Eh,Eh,Eh,Em,Em,El, 1, 1, 1]
    #   paired rhs rows:  [Rh,Rm,Rl,Rh,Rm,Rh, bh,bm,bl]
    # taylor branch rows: [1,1,1, p,p,p, p2h,p2h,p2l, p3,p3, p4]
    #   paired rhs rows:  [c0h,c0m,c0l, c1h,c1m,c1l, c2h,c2l,c2h, c3h,c3l, c4h]
    KB = 9 if use_exp else 12
    lhs_base = np.zeros((KB, 128), dtype=np.float64)
    if use_exp:
        E = np.exp(gamma * pvec / SR)
        Eh, Em, El = bsplit(E, 3)
        for i, v in enumerate([Eh, Eh, Eh, Em, Em, El]):
            lhs_base[i] = np.asarray(v, dtype=np.float64)
        lhs_base[6] = lhs_base[7] = lhs_base[8] = 1.0
    else:
        lhs_base[0] = lhs_base[1] = lhs_base[2] = 1.0
        lhs_base[3] = lhs_base[4] = lhs_base[5] = pvec
        p2h, p2l = bsplit(pvec ** 2, 2)
        lhs_base[6] = lhs_base[7] = np.asarray(p2h, np.float64)
        lhs_base[8] = np.asarray(p2l, np.float64)
        p3h = bsplit(pvec ** 3, 1)[0]
        lhs_base[9] = lhs_base[10] = np.asarray(p3h, np.float64)
        lhs_base[11] = np.asarray(bsplit(pvec ** 4, 1)[0], np.float64)
    lhs_base_bf = lhs_base.astype(ml_dtypes.bfloat16)
    lhs_base_f64 = lhs_base_bf.astype(np.float64)

    # tile col c covers samples i = 128*(c+TRIM_L) + p - r (grain-local),
    # i.e. global cols q+c with q = onset//128 + TRIM_L: the (129-GCOLS)
    # dropped edge cols carry only the Hann window's faint tails.
    TRIM_L = (129 - GCOLS) // 2
    cvec = np.arange(GCOLS, dtype=np.float64) + TRIM_L
    ncols = gpc_pad * GCOLS

    fact = [1, 1, 2, 6, 24, 120]

    cores = []
    seg_S = 8  # shared across cores (program structure is per-core anyway)
    for c in range(N_CORES):
        gsel = np.arange(c * GPC, (c + 1) * GPC)
        q = on[gsel] // 128 + TRIM_L
        order = np.argsort(q, kind="stable")
        gsel = gsel[order]
        q = q[order]
        r = on[gsel] % 128

        # strip assignment (greedy, span <= STRIP_COLS, no coverage gaps)
        strips = []  # list of [base, first_idx, last_idx, covered_end]
        base = None
        for k in range(GPC):
            qk = int(q[k])
            if (base is None or qk + GCOLS > base + STRIP_COLS
                    or qk > strips[-1][3]):
                base = qk
                strips.append([base, k, k, qk + GCOLS])
            else:
                strips[-1][2] = k
                strips[-1][3] = max(strips[-1][3], qk + GCOLS)
        # dummy (pad) grains have zero amplitude; their scatters are skipped
        # entirely (emit_scatter guards on g >= GPC), so strips only cover
        # real grains
        n_dummy = gpc_pad - GPC
        q_dummy = strips[-1][0]

        f0c = f0[gsel]
        Ac = A[gsel]

        # ideal theta model at p=0 (for the per-column base), [GPC, 129]
        # beta[g, c] = (128*c - r_g)/SR - D/2   (t at p=0)
        beta = (128.0 * cvec[None, :] - r[:, None]) / SR - GRAIN_DUR_S / 2.0
        if use_exp:
            a_g = f0c / gamma
            R_ideal = (a_g[:, None]) * np.exp(gamma * beta)
            const_ideal = np.broadcast_to(-a_g[:, None], beta.shape)
            theta0 = R_ideal + const_ideal  # theta at p=0
        else:
            coeff = np.zeros((5, GPC, GCOLS), dtype=np.float64)  # j = 0..4
            for k in range(1, 6):
                gk = gamma ** (k - 1) / fact[k]
                for j in range(0, min(k, 4) + 1):
                    binom = math.comb(k, j)
                    coeff[j] += gk * binom * beta ** (k - j) * SR ** (-j)
            coeff *= f0c[None, :, None]
            theta0 = coeff[0]

        base_c = np.round(theta0)  # folded into the const row -> |theta'| small

        # build bf16-split rhs base rows [KB, ncols]
        rhs64 = np.zeros((KB, ncols), dtype=np.float64)

        def put(row, arr):
            rhs64[row, : GPC * GCOLS] = np.asarray(arr, np.float64).reshape(-1)

        if use_exp:
            Rh, Rm, Rl = bsplit(R_ideal, 3)
            bh, bm, bl = bsplit(const_ideal - base_c, 3)
            for i, v in enumerate([Rh, Rm, Rl, Rh, Rm, Rh, bh, bm, bl]):
                put(i, v)
        else:
            c0h, c0m, c0l = bsplit(coeff[0] - base_c, 3)
            c1h, c1m, c1l = bsplit(coeff[1], 3)
            c2h, c2l = bsplit(coeff[2], 2)
            c3h, c3l = bsplit(coeff[3], 2)
            c4h = bsplit(coeff[4], 1)[0]
            for i, v in enumerate([c0h, c0m, c0l, c1h, c1m, c1l,
                                   c2h, c2l, c2h, c3h, c3l, c4h]):
                put(i, v)
        rhs_base = rhs64.astype(ml_dtypes.bfloat16)

        # device-model theta (f64 sim of the bf16 matmul), [128, ncols]
        theta = lhs_base_f64.T @ rhs_base.astype(np.float64)

        # segment hints: S partition segments, integer hint per (segment, col)
        S = seg_S
        while True:
            t3 = theta.reshape(S, 128 // S, ncols)
            mid = 0.5 * (t3.max(axis=1) + t3.min(axis=1))
            rseg = np.round(mid)  # [S, ncols] integers
            resid = np.abs(t3 - rseg[:, None, :]).max()
            if resid <= SEG_MARGIN or S >= 64:
                break
            S *= 2
        seg_S = max(seg_S, S)

        cores.append(
            dict(
                rhs_base=rhs_base,
                theta=theta,
                r=r,
                q=np.concatenate([q, np.full(n_dummy, q_dummy, dtype=np.int64)]),
                strips=strips,
                Ac=Ac,
            )
        )

    # second pass: uniform S across cores; build final lhs/rhs (+wa) arrays
    S = seg_S
    KR = KB + S
    seg = 128 // S
    lhs = np.zeros((KR, 128), dtype=np.float64)
    lhs[:KB] = lhs_base_f64
    for k in range(S):
        lhs[KB + k, k * seg:(k + 1) * seg] = 1.0
    lhs_bf = lhs.astype(ml_dtypes.bfloat16)

    resid_max = 0.0
    fallback = False
    pvec_i = np.arange(128)
    for c in range(N_CORES):
        info = cores[c]
        theta = info.pop("theta")
        t3 = theta.reshape(S, seg, ncols)
        mid = 0.5 * (t3.max(axis=1) + t3.min(axis=1))
        rseg = np.round(mid)  # [S, ncols]
        resid = np.abs(t3 - rseg[:, None, :]).max()
        resid_max = max(resid_max, resid)
        assert np.abs(rseg).max() < 250, "segment hint exceeds bf16-exact range"
        rhs = np.zeros((KR, ncols), dtype=np.float64)
        rhs[:KB] = info.pop("rhs_base").astype(np.float64)
        rhs[KB:] = -rseg
        info["rhs"] = rhs.astype(ml_dtypes.bfloat16)

        # WA table: A_g * W(i), i = 128*c + p - r_g.  The sample-range mask is
        # dropped: outside [0, GRAIN_N) the Hann window value is ~(pi*i/N)^2
        # <= 6e-4, negligible vs the 2e-2 error budget.
        r = info.pop("r")
        Ac = info.pop("Ac")
        i_idx = (128 * cvec[None, None, :] + pvec_i[None, :, None]
                 - r[:, None, None])  # [GPC, 128, 129]
        W = np.sin(np.pi * i_idx / GRAIN_N) ** 2
        WA = (W * Ac[:, None, None]).transpose(1, 0, 2).reshape(128, GPC * GCOLS)
        wa_full = np.zeros((128, ncols), dtype=np.float64)
        wa_full[:, : GPC * GCOLS] = WA
        info["wa"] = wa_full.astype(ml_dtypes.bfloat16)

    if resid_max > SEG_MARGIN:
        fallback = True  # extreme chirp slope: per-element hints needed

    meta = dict(
        lhs=lhs_bf,
        gpc_pad=gpc_pad,
        n_batches=n_batches,
        use_exp=use_exp,
        gamma=gamma,
        ncols=ncols,
        KR=KR,
        fallback=fallback,
        resid=resid_max,
    )
    return cores, meta


def _build_program(cores, meta, single_core=False):
    import concourse.bacc as bacc
    import concourse.bass as bass
    import concourse.tile as tile
    import concourse.mybir as mybir
    from concourse import bass_utils  # noqa: F401

    ncols = meta["ncols"]
    n_batches = meta["n_batches"]
    KR = meta["KR"]

    nc = bacc.Bacc("TRN2", target_bir_lowering=False, debug=False,
                   num_devices=1 if single_core else N_CORES)
    f32 = mybir.dt.float32
    bf16 = mybir.dt.bfloat16

    d_lhs = nc.dram_tensor("lhs", [KR, 128], bf16, kind="ExternalInput").ap()
    d_rhs = nc.dram_tensor("rhs", [KR, ncols], bf16, kind="ExternalInput").ap()
    d_wa = nc.dram_tensor("wa", [128, ncols], bf16, kind="ExternalInput").ap()
    d_iden = nc.dram_tensor("iden", [128, 128], bf16, kind="ExternalInput").ap()
    d_out = nc.dram_tensor("out", [65536], f32, kind="ExternalOutput").ap()

    AF = mybir.ActivationFunctionType
    ALU = mybir.AluOpType
    TWO_PI = float(2.0 * np.pi)

    with tile.TileContext(nc) as tc, ExitStack() as octx:
        outer = octx.enter_context(tc.tile_pool(name="outer", bufs=1))
        acc = outer.tile([128, ACC_PAD_COLS], f32)
        # memset on DVE (idle at startup): on Pool it would sit ahead of the
        # rhs SWDGE descriptor generations and delay the first theta by ~4us
        nc.vector.memset(acc[:], 0.0)
        lhs_t = outer.tile([KR, 128], bf16)
        nc.sync.dma_start(lhs_t[:], d_lhs[:])  # tiny, ahead of rhs0 on SP
        iden = outer.tile([128, 128], bf16)
        # iden's DMA is issued inside the core body after the first input
        # chunk: it is first read only at the first scatter (~10us in) and
        # must not delay rhs chunk 0 on the DMA engines.

        if not single_core:
            dram = octx.enter_context(
                tc.tile_pool(name="dram", bufs=1, space="DRAM"))
            b_in = dram.tile([128, ACC_COLS], f32)

        def emit_core_body(core):
            """Returns nothing; flushes final acc columns eagerly into the
            reduce input (SPMD) or the output (single-core estimate)."""
            info = cores[core]
            q = info["q"]
            strips = info["strips"]

            if single_core:
                flush_dst = d_full
            else:
                flush_dst = b_in[:]

            with ExitStack() as ctx:
                rhsp = ctx.enter_context(
                    tc.tile_pool(name=f"rhs{core}", bufs=1))
                wap = ctx.enter_context(tc.tile_pool(name=f"wap{core}", bufs=5))
                sp = ctx.enter_context(tc.tile_pool(name=f"sp{core}", bufs=3))
                vp = ctx.enter_context(
                    tc.tile_pool(name=f"vp{core}", bufs=SCATTER_LAG + 2))
                thp = ctx.enter_context(
                    tc.tile_pool(name=f"th{core}", bufs=2, space="PSUM"))
                stp = ctx.enter_context(
                    tc.tile_pool(name=f"st{core}", bufs=2, space="PSUM"))

                # strip state machine across batches
                strip_iter = iter(strips)
                cur = next(strip_iter)
                cur_tile = None
                flushed = 0  # acc cols already written out
                NB = BATCH * GCOLS  # 1161

                def flush_to(limit, force=False):
                    """DMA-out final acc cols [flushed, limit).  Issued from
                    the otherwise-idle Pool queue: a flush's sem-wait on
                    pending strip evacs must not delay the SP-issued input
                    DMAs."""
                    nonlocal flushed
                    lim = min(limit, ACC_COLS)
                    if lim - flushed >= FLUSH_MIN or (force and lim > flushed):
                        nc.gpsimd.dma_start(
                            flush_dst[:, flushed:lim], acc[:, flushed:lim])
                        flushed = lim

                def emit_scatter(g0, t_v, ng=BATCH, voff=0):
                    nonlocal cur, cur_tile
                    for j in range(ng):
                        g = g0 + j
                        if g >= GPC:  # zero-amplitude pad grain
                            continue
                        # open new strip?
                        if g > cur[2]:
                            # evacuate finished strip (covered span)
                            w = cur[3] - cur[0]
                            nc.vector.tensor_add(
                                acc[:, cur[0]:cur[0] + w],
                                cur_tile[:, :w],
                                acc[:, cur[0]:cur[0] + w],
                            )
                            cur = next(strip_iter)
                            cur_tile = None
                            # at the last strip, flush everything below its
                            # base now so the tail flush only covers its span
                            flush_to(cur[0], force=cur is strips[-1])
                        first = cur_tile is None
                        if first:
                            cur_tile = stp.tile(
                                [128, STRIP_COLS], f32, tag="strip")
                        off = int(q[g]) - cur[0]
                        last = g == cur[2]
                        nc.tensor.matmul(
                            cur_tile[:, off:off + GCOLS],
                            iden[:],
                            t_v[:, voff + j * GCOLS:voff + (j + 1) * GCOLS],
                            start=first, stop=last,
                        )

                # software pipeline: scatter runs SCATTER_LAG batches behind
                # the theta->sin->window chain so PE's in-order queue never
                # blocks upcoming theta matmuls on a v that was only just
                # produced (the ACT->DVE->v chain is ~1.2us deep).
                from collections import deque
                pending = deque()  # (g0, t_v) awaiting scatter

                # Input DMA routing: all rhs chunks are issued upfront from
                # the Pool queue (SWDGE) so they are never queued behind a wa
                # pool-slot wait -- if they were, the Tile scheduler's own
                # pipeline sim would see theta matmuls blocked on rhs and lock
                # a scatter-before-theta PE order that stalls ACT at runtime.
                # wa chunks stream on SP, self-paced by their pool slots.
                n_chunks = (n_batches + DMA_B - 1) // DMA_B
                rhs_all = rhsp.tile([KR, ncols], bf16, tag="rhs")
                wa_tiles = []

                def emit_rhs_dma(k):
                    col0 = k * DMA_B * NB
                    W2 = min(DMA_B, n_batches - k * DMA_B) * NB
                    # chunk 0 from SP (fast HWDGE, ahead of wa0 on the DMA
                    # engines -- it gates the first theta); the rest from
                    # Pool/SWDGE so they never sit behind a wa slot wait
                    eng = nc.sync if k == 0 else nc.gpsimd
                    eng.dma_start(
                        rhs_all[:, col0:col0 + W2], d_rhs[:, col0:col0 + W2])

                def emit_wa_dma(k):
                    col0 = k * DMA_B * NB
                    W2 = min(DMA_B, n_batches - k * DMA_B) * NB
                    t = wap.tile([128, DMA_B * NB], bf16, tag="wa")
                    nc.sync.dma_start(t[:, :W2], d_wa[:, col0:col0 + W2])
                    wa_tiles.append(t)

                for k in range(n_chunks):
                    emit_rhs_dma(k)
                for k in range(n_chunks):
                    emit_wa_dma(k)
                    if k == 0:
                        nc.sync.dma_start(iden[:], d_iden[:])

                for b2 in range(0, n_batches, DMA_B):
                    nb2 = min(DMA_B, n_batches - b2)
                    t_wa2 = wa_tiles[b2 // DMA_B]

                    for b in range(b2, b2 + nb2):
                        part = (b - b2) * NB
                        g0 = b * BATCH
                        t_rhs = rhs_all[:, b * NB:(b + 1) * NB]

                        # the last batch runs at GPB-grain granularity so the
                        # end-of-program ACT->mul->scatter chain is 1/3 the
                        # depth; fully-dummy sub-units are skipped outright
                        lastb = b == n_batches - 1
                        nsub = -(-(GPC - g0) // GPB) if lastb else 3
                        nsub = max(1, min(3, nsub))

                        th = thp.tile([128, 3 * 512], f32, tag="th")
                        # priority-bias the theta matmuls ahead of the
                        # previous batch's scatter matmuls so PE's in-order
                        # dispatch never blocks them behind a v-wait.
                        with tc.high_priority(offset=16):
                            for m in range(nsub):
                                sl = slice(
                                    m * GPB * GCOLS, (m + 1) * GPB * GCOLS)
                                nc.tensor.matmul(
                                    th[:, m * 512: m * 512 + GPB * GCOLS],
                                    lhs_t[:],
                                    t_rhs[:, sl],
                                    start=True, stop=True,
                                )
                        t_s = sp.tile([128, NB], bf16, tag="s")
                        t_v = vp.tile([128, NB], bf16, tag="v")
                        if not lastb:
                            th3 = th[:].rearrange(
                                "p (b x) -> p b x", b=3)[:, :, :GPB * GCOLS]
                            s3 = t_s[:].rearrange("p (b x) -> p b x", b=3)
                            nc.scalar.activation(s3, th3, AF.Sin, scale=TWO_PI)
                            nc.vector.tensor_mul(
                                t_v[:], t_s[:], t_wa2[:, part:part + NB])
                            pending.append((g0, t_v, BATCH, 0))
                            if len(pending) > SCATTER_LAG:
                                emit_scatter(*pending.popleft())
                        else:
                            U = GPB * GCOLS
                            for m in range(nsub):
                                nc.scalar.activation(
                                    t_s[:, m * U:(m + 1) * U],
                                    th[:, m * 512: m * 512 + U],
                                    AF.Sin, scale=TWO_PI)
                                nc.vector.tensor_mul(
                                    t_v[:, m * U:(m + 1) * U],
                                    t_s[:, m * U:(m + 1) * U],
                                    t_wa2[:, part + m * U:part + (m + 1) * U])
                                pending.append((g0 + m * GPB, t_v, GPB, m * U))
                                if len(pending) > SCATTER_LAG:
                                    emit_scatter(*pending.popleft())
                head = 0
                while pending:
                    if (len(pending) == 1 and cur is strips[-1]
                            and cur_tile is not None):
                        # grains are onset-sorted, so the final strip's head
                        # cols are already complete: evacuate + flush them now
                        # (on SP's HWDGE) so the post-drain tail only covers
                        # the last grains' span.
                        g0n, _, ngn, _ = pending[0]
                        lo = min(int(q[g])
                                 for g in range(g0n, min(g0n + ngn, GPC)))
                        head = max(0, lo - cur[0])
                        if head >= 128:
                            nc.vector.tensor_add(
                                acc[:, cur[0]:cur[0] + head],
                                cur_tile[:, :head],
                                acc[:, cur[0]:cur[0] + head])
                            nc.sync.dma_start(
                                flush_dst[:, flushed:cur[0] + head],
                                acc[:, flushed:cur[0] + head])
                            flushed = cur[0] + head
                        else:
                            head = 0
                    emit_scatter(*pending.popleft())
                # final strip tail: evacuate the remainder and flush it
                base, w = cur[0], cur[3] - cur[0]
                nc.vector.tensor_add(
                    acc[:, base + head:base + w], cur_tile[:, head:w],
                    acc[:, base + head:base + w])
                nc.gpsimd.dma_start(
                    flush_dst[:, flushed:ACC_COLS],
                    acc[:, flushed:ACC_COLS])
                del wa_tiles[:]

        if single_core:
            d_full = nc.dram_tensor(
                "full", [128, ACC_COLS], f32, kind="ExternalOutput").ap()
            emit_core_body(0)
        else:
            pid = nc.partition_id()
            for core in range(N_CORES):
                with tc.If(pid == core):
                    emit_core_body(core)

            # ---- shared epilog: reduce, normalize, output ----
            b_rs = dram.tile([16, ACC_COLS], f32)
            nc.gpsimd.collective_compute(
                "ReduceScatter",
                mybir.AluOpType.add,
                replica_groups=[list(range(N_CORES))],
                ins=[b_in[:].opt()],
                outs=[b_rs[:].opt()],
            )
            red = outer.tile([128, 512], f32)
            nc.sync.dma_start(
                red[:],
                b_rs[:].rearrange("a b -> (a b)").rearrange(
                    "(p c) -> p c", p=128))

            # sum of squares of the local chunk
            scr = outer.tile([128, 512], f32)
            sqcol = outer.tile([128, 1], f32)
            nc.scalar.activation(scr[:], red[:], AF.Square, accum_out=sqcol[:])
            ones = outer.tile([128, 128], f32)
            nc.vector.memset(ones[:], 1.0)
            psq = octx.enter_context(tc.tile_pool(name="psq", bufs=1, space="PSUM"))
            ps_s = psq.tile([1, 128], f32)
            nc.tensor.matmul(ps_s[:], sqcol[:], ones[:], start=True, stop=True)
            ssq = outer.tile([1, 128], f32)
            nc.vector.tensor_copy(ssq[:], ps_s[:])

            b_s1 = dram.tile([1, 128], f32)
            b_s2 = dram.tile([1, 128], f32)
            nc.sync.dma_start(b_s1[:], ssq[:])
            nc.gpsimd.collective_compute(
                "AllReduce",
                mybir.AluOpType.add,
                replica_groups=[list(range(N_CORES))],
                ins=[b_s1[:].opt()],
                outs=[b_s2[:].opt()],
            )
            gsq = outer.tile([1, 1], f32)
            nc.sync.dma_start(gsq[:], b_s2[:, 0:1])

            # rscale = rsqrt(gsq) with one Newton refinement
            nrm = outer.tile([1, 1], f32)
            nc.scalar.activation(nrm[:], gsq[:], AF.Sqrt)
            z0 = outer.tile([1, 1], f32)
            nc.vector.reciprocal(z0[:], nrm[:])
            z2 = outer.tile([1, 1], f32)
            nc.vector.tensor_mul(z2[:], z0[:], z0[:])
            t2 = outer.tile([1, 1], f32)
            nc.vector.tensor_mul(t2[:], z2[:], gsq[:])
            t3 = outer.tile([1, 1], f32)
            nc.vector.tensor_scalar(t3[:], t2[:], -0.5, 1.5, ALU.mult, ALU.add)
            z1 = outer.tile([1, 1], f32)
            nc.vector.tensor_mul(z1[:], z0[:], t3[:])

            # broadcast to 128 partitions via DRAM bounce
            b_z = dram.tile([1, 1], f32)
            nc.sync.dma_start(b_z[:], z1[:])
            zb = outer.tile([128, 1], f32)
            bz_ap = b_z[:]
            bcast = bass.AP(tensor=bz_ap.tensor, offset=bz_ap.offset,
                            ap=[[0, 128], [1, 1]])
            nc.sync.dma_start(zb[:], bcast)

            outt = outer.tile([128, 512], f32)
            nc.vector.tensor_scalar(outt[:], red[:], zb[:], None, ALU.mult)
            nc.sync.dma_start(
                d_out.rearrange("(p c) -> p c", p=128), outt[:])

    nc.compile()
    return nc


def estimate_hw_time_ns(theta_density, theta_slope, f0_freqs_hz, onsets):
    """Cost-model (TimelineSim) estimate of one core's execution, ns.

    Single-core variant: core 0's synthesis+scatter+evac plus the 2MB
    accumulator DMA-out (standing in for the ReduceScatter contribution).
    """
    from concourse.timeline_sim import TimelineSim

    cores, meta = _host_prep(theta_density, theta_slope, f0_freqs_hz, onsets)
    nc = _build_program(cores, meta, single_core=True)
    ts = TimelineSim(nc)
    ts.simulate()
    return float(ts.time)


def kernel(theta_density, theta_slope, f0_freqs_hz, onsets):
    import ml_dtypes
    from concourse import bass_utils

    cores, meta = _host_prep(theta_density, theta_slope, f0_freqs_hz, onsets)
    nc = _build_program(cores, meta)

    iden = np.eye(128, dtype=np.float32).astype(ml_dtypes.bfloat16)
    in_maps = []
    for c in range(N_CORES):
        info = cores[c]
        in_maps.append(
            dict(
                lhs=meta["lhs"],
                rhs=info["rhs"],
                wa=info["wa"],
                iden=iden,
            )
        )
    res = bass_utils.run_bass_kernel_spmd(
        nc, in_maps, core_ids=list(range(N_CORES)))

    X = np.zeros((ACC_COLS, 128), dtype=np.float32)
    for c in range(N_CORES):
        chunk = res.results[c]["out"].reshape(16, ACC_COLS)
        X[:, 16 * c:16 * (c + 1)] = chunk.T
    return X.reshape(-1).astype(np.float32)


if __name__ == "__main__":
    rng = np.random.default_rng(0)
    out = kernel(
        np.float32(0.5), np.float32(0.3),
        np.exp(rng.uniform(np.log(F0_MIN), np.log(F0_MAX), N_GRAINS)).astype(np.float32),
        rng.integers(0, N_SAMPLES - GRAIN_N, N_GRAINS).astype(np.int32),
    )
    print(out.shape, out[:8], np.linalg.norm(out))



# revision 39
# speedup vs baseline: 1.2544x; 1.0296x over previous
"""ChirpTextureSynth Trainium2 kernel.

Synthesizes 4096 windowed chirp grains (16384 samples each), scatter-adds
them at per-grain onsets into a 524288-sample signal, L2-normalizes.

Strategy (8 NeuronCores, data-parallel over grains, 512 grains/core):
 - Output accumulator layout: sample s -> (partition p = s % 128, col = s // 128).
   A grain at onset o occupies cols [o//128, o//128 + 129) on all partitions
   (onsets never wrap: o < N_SAMPLES - GRAIN_N).
 - Sine argument in CYCLES: theta(p,c) = f0*phase(t), t = i/SR - D/2,
   i = 128*c + p - (o % 128). theta is low-rank separable in (p, c):
   exp branch  : theta = a*E(p)*F(c) + b,  E(p)=exp(g*p/SR)
   taylor branch (|g| < 0.7): theta = sum_j coeff_j(c) * p^j, j=0..4
 - Range reduction is folded INTO the theta matmul: S piecewise-constant
   partition-segment hint rows (indicator lhsT rows x integer bf16 rhs rows)
   subtract round(theta) per segment, leaving |u| <= ~0.58.  The ACT Sin
   spline (scale=2pi) is accurate to ~3e-4 out to |u|=0.58, so ONE K<=128
   matmul per 3 grains produces ready-to-sin u tiles in PSUM (f32).
   Fallback for extreme chirp slopes (segment residual too big): per-element
   int8 hint plane applied via identity matmul / DVE, as before.
 - ACT Sin(scale=2pi) evaluates sin(2*pi*u) -> bf16 SBUF.
 - Window*amp (bf16 host table WA) applied on DVE: v = s * WA (2x bf16).
 - Scatter: per-grain matmul with identity weights accumulates v into a
   PSUM "strip" bank (512 cols); strips follow onset-sorted grains; DVE
   evacuates each strip into the SBUF accumulator.  Columns left of the
   next strip's base are final and are streamed out to DRAM eagerly.
 - Per-core instruction streams differ (grain offsets are immediates), so the
   program has 8 tc.If(partition_id == c) branches; inputs differ per core.
 - Reduction: ReduceScatter (128x4096 f32) + scalar AllReduce of sum-of-
   squares; each core normalizes and outputs its 1/8 chunk; host reassembles.
"""

import math
from contextlib import ExitStack

import numpy as np

SR = 44100.0
N_SAMPLES = 524288
N_GRAINS = 4096
GRAIN_N = 16384
F0_MIN = 32.7
F0_MAX = 523.25
Q = 12
HOP_LEN = 256
GRAIN_DUR_S = GRAIN_N / SR
N_CORES = 8
GPC = N_GRAINS // N_CORES  # grains per core (512)

ACC_COLS = N_SAMPLES // 128        # 4096
ACC_PAD_COLS = ACC_COLS + 384      # strip overhang room
GCOLS = 113                        # cols per grain tile; (129-GCOLS)/2 cols
                                   # trimmed per edge where the Hann window is
                                   # tiny (rel-L2 cost ~10*f^2.5, f=trim frac)
BATCH = 12                         # grains per compute batch (3 psum banks)
GPB = 4                            # grains per theta-matmul (508 cols <= 512)
TAYLOR_CUT = 0.7                   # |gamma| below which the poly branch is used
STRIP_COLS = 512
SEG_MARGIN = 0.65                  # max |u| the Sin spline tolerates (~5e-3)
DMA_B = 2                          # batches per input DMA chunk
DMA_PRE = 5                        # chunks interleaved rhs/wa at the start
FLUSH_MIN = 384                    # min final cols before an output flush
SCATTER_LAG = 4                    # batches the scatter trails the sin chain


def _host_prep(theta_density, theta_slope, f0_freqs_hz, onsets):
    """All host-side precompute. Returns per-core input arrays + metadata."""
    td = float(np.float32(theta_density))
    ts = float(np.float32(theta_slope))
    f0 = np.asarray(f0_freqs_hz, dtype=np.float64)
    on = np.asarray(onsets, dtype=np.int64)

    # per-grain amplitudes (matches reference, f64 is fine vs f32 ref)
    gi = np.arange(N_GRAINS, dtype=np.float64)
    offset = 0.25 * td + 0.75 * td * td
    sig_op = (1.0 - td) * N_GRAINS * (gi / N_GRAINS - offset)
    amps = 0.5 * (1.0 - np.tanh(sig_op))  # = 1 - sigmoid(2*sig_op), stable
    amps = amps / amps.max()
    A = amps / np.sqrt(f0)

    typical_slope = SR / (Q * HOP_LEN)
    gamma = math.tan(ts * math.pi / 2.0) * typical_slope / 4.0

    use_exp = abs(gamma) >= TAYLOR_CUT

    # padded grain count per core -> multiple of BATCH
    gpc_pad = ((GPC + BATCH - 1) // BATCH) * BATCH   # 513
    n_batches = gpc_pad // BATCH

    import ml_dtypes

    def bsplit(x, n):
        """Split f64 array into n bf16 parts summing to ~x (24 bits for n=3)."""
        parts = []
        rem = np.array(x, dtype=np.float64, copy=True)
        for _ in range(n):
            h = rem.astype(ml_dtypes.bfloat16)
            parts.append(h)
            rem = rem - h.astype(np.float64)
        return parts

    pvec = np.arange(128, dtype=np.float64)
    # base lhsT rows [KB, 128] in bf16; theta matmul runs at bf16 rate.
    # exp branch rows:    [Eh,Eh,Eh,Em,Em,El, 1, 1, 1]
    #   paired rhs rows:  [Rh,Rm,Rl,Rh,Rm,Rh, bh,bm,bl]
    # taylor branch rows: [1,1,1, p,p,p, p2h,p2h,p2l, p3,p3, p4]
    #   paired rhs rows:  [c0h,c0m,c0l, c1h,c1m,c1l, c2h,c2l,c2h, c3h,c3l, c4h]
    KB = 9 if use_exp else 12
    lhs_base = np.zeros((KB, 128), dtype=np.float64)
    if use_exp:
        E = np.exp(gamma * pvec / SR)
        Eh, Em, El = bsplit(E, 3)
        for i, v in enumerate([Eh, Eh, Eh, Em, Em, El]):
            lhs_base[i] = np.asarray(v, dtype=np.float64)
        lhs_base[6] = lhs_base[7] = lhs_base[8] = 1.0
    else:
        lhs_base[0] = lhs_base[1] = lhs_base[2] = 1.0
        lhs_base[3] = lhs_base[4] = lhs_base[5] = pvec
        p2h, p2l = bsplit(pvec ** 2, 2)
        lhs_base[6] = lhs_base[7] = np.asarray(p2h, np.float64)
        lhs_base[8] = np.asarray(p2l, np.float64)
        p3h = bsplit(pvec ** 3, 1)[0]
        lhs_base[9] = lhs_base[10] = np.asarray(p3h, np.float64)
        lhs_base[11] = np.asarray(bsplit(pvec ** 4, 1)[0], np.float64)
    lhs_base_bf = lhs_base.astype(ml_dtypes.bfloat16)
    lhs_base_f64 = lhs_base_bf.astype(np.float64)

    # tile col c covers samples i = 128*(c+TRIM_L) + p - r (grain-local),
    # i.e. global cols q+c with q = onset//128 + TRIM_L: the (129-GCOLS)
    # dropped edge cols carry only the Hann window's faint tails.
    TRIM_L = (129 - GCOLS) // 2
    cvec = np.arange(GCOLS, dtype=np.float64) + TRIM_L
    ncols = gpc_pad * GCOLS

    fact = [1, 1, 2, 6, 24, 120]

    cores = []
    seg_S = 8  # shared across cores (program structure is per-core anyway)
    for c in range(N_CORES):
        gsel = np.arange(c * GPC, (c + 1) * GPC)
        q = on[gsel] // 128 + TRIM_L
        order = np.argsort(q, kind="stable")
        gsel = gsel[order]
        q = q[order]
        r = on[gsel] % 128

        # strip assignment (greedy, span <= STRIP_COLS, no coverage gaps)
        strips = []  # list of [base, first_idx, last_idx, covered_end]
        base = None
        for k in range(GPC):
            qk = int(q[k])
            if (base is None or qk + GCOLS > base + STRIP_COLS
                    or qk > strips[-1][3]):
                base = qk
                strips.append([base, k, k, qk + GCOLS])
            else:
                strips[-1][2] = k
                strips[-1][3] = max(strips[-1][3], qk + GCOLS)
        # split the trailing strip so the final strip holds only the last few
        # grains: the (large) penultimate strip then evacuates and flushes
        # during the drain, leaving a short end-of-program chain
        split_at = GPC - GPB
        if strips[-1][1] < split_at:
            old = strips[-1]
            cov = int(max(q[old[1]:split_at])) + GCOLS
            strips[-1] = [old[0], old[1], split_at - 1, cov]
            strips.append([int(q[split_at]), split_at, GPC - 1, old[3]])
        # dummy (pad) grains have zero amplitude; their scatters are skipped
        # entirely (emit_scatter guards on g >= GPC), so strips only cover
        # real grains
        n_dummy = gpc_pad - GPC
        q_dummy = strips[-1][0]

        f0c = f0[gsel]
        Ac = A[gsel]

        # ideal theta model at p=0 (for the per-column base), [GPC, 129]
        # beta[g, c] = (128*c - r_g)/SR - D/2   (t at p=0)
        beta = (128.0 * cvec[None, :] - r[:, None]) / SR - GRAIN_DUR_S / 2.0
        if use_exp:
            a_g = f0c / gamma
            R_ideal = (a_g[:, None]) * np.exp(gamma * beta)
            const_ideal = np.broadcast_to(-a_g[:, None], beta.shape)
            theta0 = R_ideal + const_ideal  # theta at p=0
        else:
            coeff = np.zeros((5, GPC, GCOLS), dtype=np.float64)  # j = 0..4
            for k in range(1, 6):
                gk = gamma ** (k - 1) / fact[k]
                for j in range(0, min(k, 4) + 1):
                    binom = math.comb(k, j)
                    coeff[j] += gk * binom * beta ** (k - j) * SR ** (-j)
            coeff *= f0c[None, :, None]
            theta0 = coeff[0]

        base_c = np.round(theta0)  # folded into the const row -> |theta'| small

        # build bf16-split rhs base rows [KB, ncols]
        rhs64 = np.zeros((KB, ncols), dtype=np.float64)

        def put(row, arr):
            rhs64[row, : GPC * GCOLS] = np.asarray(arr, np.float64).reshape(-1)

        if use_exp:
            Rh, Rm, Rl = bsplit(R_ideal, 3)
            bh, bm, bl = bsplit(const_ideal - base_c, 3)
            for i, v in enumerate([Rh, Rm, Rl, Rh, Rm, Rh, bh, bm, bl]):
                put(i, v)
        else:
            c0h, c0m, c0l = bsplit(coeff[0] - base_c, 3)
            c1h, c1m, c1l = bsplit(coeff[1], 3)
            c2h, c2l = bsplit(coeff[2], 2)
            c3h, c3l = bsplit(coeff[3], 2)
            c4h = bsplit(coeff[4], 1)[0]
            for i, v in enumerate([c0h, c0m, c0l, c1h, c1m, c1l,
                                   c2h, c2l, c2h, c3h, c3l, c4h]):
                put(i, v)
        rhs_base = rhs64.astype(ml_dtypes.bfloat16)

        # device-model theta (f64 sim of the bf16 matmul), [128, ncols]
        theta = lhs_base_f64.T @ rhs_base.astype(np.float64)

        # segment hints: S partition segments, integer hint per (segment, col)
        S = seg_S
        while True:
            t3 = theta.reshape(S, 128 // S, ncols)
            mid = 0.5 * (t3.max(axis=1) + t3.min(axis=1))
            rseg = np.round(mid)  # [S, ncols] integers
            resid = np.abs(t3 - rseg[:, None, :]).max()
            if resid <= SEG_MARGIN or S >= 64:
                break
            S *= 2
        seg_S = max(seg_S, S)

        cores.append(
            dict(
                rhs_base=rhs_base,
                theta=theta,
                r=r,
                q=np.concatenate([q, np.full(n_dummy, q_dummy, dtype=np.int64)]),
                strips=strips,
                Ac=Ac,
            )
        )

    # second pass: uniform S across cores; build final lhs/rhs (+wa) arrays
    S = seg_S
    KR = KB + S
    seg = 128 // S
    lhs = np.zeros((KR, 128), dtype=np.float64)
    lhs[:KB] = lhs_base_f64
    for k in range(S):
        lhs[KB + k, k * seg:(k + 1) * seg] = 1.0
    lhs_bf = lhs.astype(ml_dtypes.bfloat16)

    resid_max = 0.0
    fallback = False
    pvec_i = np.arange(128)
    for c in range(N_CORES):
        info = cores[c]
        theta = info.pop("theta")
        t3 = theta.reshape(S, seg, ncols)
        mid = 0.5 * (t3.max(axis=1) + t3.min(axis=1))
        rseg = np.round(mid)  # [S, ncols]
        resid = np.abs(t3 - rseg[:, None, :]).max()
        resid_max = max(resid_max, resid)
        assert np.abs(rseg).max() < 250, "segment hint exceeds bf16-exact range"
        rhs = np.zeros((KR, ncols), dtype=np.float64)
        rhs[:KB] = info.pop("rhs_base").astype(np.float64)
        rhs[KB:] = -rseg
        info["rhs"] = rhs.astype(ml_dtypes.bfloat16)

        # WA table: A_g * W(i), i = 128*c + p - r_g.  The sample-range mask is
        # dropped: outside [0, GRAIN_N) the Hann window value is ~(pi*i/N)^2
        # <= 6e-4, negligible vs the 2e-2 error budget.
        r = info.pop("r")
        Ac = info.pop("Ac")
        i_idx = (128 * cvec[None, None, :] + pvec_i[None, :, None]
                 - r[:, None, None])  # [GPC, 128, 129]
        W = np.sin(np.pi * i_idx / GRAIN_N) ** 2
        WA = (W * Ac[:, None, None]).transpose(1, 0, 2).reshape(128, GPC * GCOLS)
        wa_full = np.zeros((128, ncols), dtype=np.float64)
        wa_full[:, : GPC * GCOLS] = WA
        info["wa"] = wa_full.astype(ml_dtypes.bfloat16)

    if resid_max > SEG_MARGIN:
        fallback = True  # extreme chirp slope: per-element hints needed

    meta = dict(
        lhs=lhs_bf,
        gpc_pad=gpc_pad,
        n_batches=n_batches,
        use_exp=use_exp,
        gamma=gamma,
        ncols=ncols,
        KR=KR,
        fallback=fallback,
        resid=resid_max,
    )
    return cores, meta


def _build_program(cores, meta, single_core=False):
    import concourse.bacc as bacc
    import concourse.bass as bass
    import concourse.tile as tile
    import concourse.mybir as mybir
    from concourse import bass_utils  # noqa: F401

    ncols = meta["ncols"]
    n_batches = meta["n_batches"]
    KR = meta["KR"]

    nc = bacc.Bacc("TRN2", target_bir_lowering=False, debug=False,
                   num_devices=1 if single_core else N_CORES)
    f32 = mybir.dt.float32
    bf16 = mybir.dt.bfloat16

    d_lhs = nc.dram_tensor("lhs", [KR, 128], bf16, kind="ExternalInput").ap()
    d_rhs = nc.dram_tensor("rhs", [KR, ncols], bf16, kind="ExternalInput").ap()
    d_wa = nc.dram_tensor("wa", [128, ncols], bf16, kind="ExternalInput").ap()
    d_iden = nc.dram_tensor("iden", [128, 128], bf16, kind="ExternalInput").ap()
    d_out = nc.dram_tensor("out", [65536], f32, kind="ExternalOutput").ap()

    AF = mybir.ActivationFunctionType
    ALU = mybir.AluOpType
    TWO_PI = float(2.0 * np.pi)

    with tile.TileContext(nc) as tc, ExitStack() as octx:
        outer = octx.enter_context(tc.tile_pool(name="outer", bufs=1))
        # zero tile fueling PE warm-up matmuls (see emit_core_body)
        wz = outer.tile([128, 128], bf16)
        nc.vector.memset(wz[:], 0.0)
        acc = outer.tile([128, ACC_PAD_COLS], f32)
        # memset on DVE (idle at startup): on Pool it would sit ahead of the
        # rhs SWDGE descriptor generations and delay the first theta by ~4us
        nc.vector.memset(acc[:], 0.0)
        lhs_t = outer.tile([KR, 128], bf16)
        # lhs goes out on Pool's DGE so rhs chunk 0 is SP's first issue --
        # the two tiny transfers then overlap instead of pacing 625ns apart
        nc.gpsimd.dma_start(lhs_t[:], d_lhs[:])
        iden = outer.tile([128, 128], bf16)
        # iden's DMA is issued inside the core body after the first input
        # chunk: it is first read only at the first scatter (~10us in) and
        # must not delay rhs chunk 0 on the DMA engines.

        if not single_core:
            dram = octx.enter_context(
                tc.tile_pool(name="dram", bufs=1, space="DRAM"))
            b_in = dram.tile([128, ACC_COLS], f32)

        def emit_core_body(core):
            """Returns nothing; flushes final acc columns eagerly into the
            reduce input (SPMD) or the output (single-core estimate)."""
            info = cores[core]
            q = info["q"]
            strips = info["strips"]

            if single_core:
                flush_dst = d_full
            else:
                flush_dst = b_in[:]

            with ExitStack() as ctx:
                rhsp = ctx.enter_context(
                    tc.tile_pool(name=f"rhs{core}", bufs=1))
                wap = ctx.enter_context(tc.tile_pool(name=f"wap{core}", bufs=5))
                sp = ctx.enter_context(tc.tile_pool(name=f"sp{core}", bufs=3))
                vp = ctx.enter_context(
                    tc.tile_pool(name=f"vp{core}", bufs=SCATTER_LAG + 2))
                thp = ctx.enter_context(
                    tc.tile_pool(name=f"th{core}", bufs=2, space="PSUM"))
                stp = ctx.enter_context(
                    tc.tile_pool(name=f"st{core}", bufs=2, space="PSUM"))

                # PE warm-up: ~2us of throwaway matmuls on zeros so the PE
                # p-state ramp is past its low tier when rhs chunk 0 lands
                wt = stp.tile([128, STRIP_COLS], f32, tag="strip")
                for _ in range(18):
                    nc.tensor.matmul(
                        wt[:, :128], wz[:], wz[:], start=True, stop=True)

                # strip state machine across batches
                strip_iter = iter(strips)
                cur = next(strip_iter)
                cur_tile = None
                flushed = 0  # acc cols already written out
                NB = BATCH * GCOLS  # 1161

                def flush_to(limit, force=False):
                    """DMA-out final acc cols [flushed, limit).  Issued from
                    the otherwise-idle Pool queue: a flush's sem-wait on
                    pending strip evacs must not delay the SP-issued input
                    DMAs."""
                    nonlocal flushed
                    lim = min(limit, ACC_COLS)
                    if lim - flushed >= FLUSH_MIN or (force and lim > flushed):
                        # the forced (drain-time) flush takes SP's HWDGE: SP
                        # is idle then and its issue path is ~1us faster than
                        # Pool's SWDGE generation
                        eng = nc.sync if force else nc.gpsimd
                        eng.dma_start(
                            flush_dst[:, flushed:lim], acc[:, flushed:lim])
                        flushed = lim

                def emit_scatter(g0, t_v, ng=BATCH, voff=0):
                    nonlocal cur, cur_tile
                    for j in range(ng):
                        g = g0 + j
                        if g >= GPC:  # zero-amplitude pad grain
                            continue
                        # open new strip?
                        if g > cur[2]:
                            # evacuate finished strip (covered span)
                            w = cur[3] - cur[0]
                            nc.vector.tensor_add(
                                acc[:, cur[0]:cur[0] + w],
                                cur_tile[:, :w],
                                acc[:, cur[0]:cur[0] + w],
                            )
                            cur = next(strip_iter)
                            cur_tile = None
                            # at the last strip, flush everything below its
                            # base now so the tail flush only covers its span
                            flush_to(cur[0], force=cur is strips[-1])
                        first = cur_tile is None
                        if first:
                            cur_tile = stp.tile(
                                [128, STRIP_COLS], f32, tag="strip")
                        off = int(q[g]) - cur[0]
                        last = g == cur[2]
                        nc.tensor.matmul(
                            cur_tile[:, off:off + GCOLS],
                            iden[:],
                            t_v[:, voff + j * GCOLS:voff + (j + 1) * GCOLS],
                            start=first, stop=last,
                        )

                # software pipeline: scatter runs SCATTER_LAG batches behind
                # the theta->sin->window chain so PE's in-order queue never
                # blocks upcoming theta matmuls on a v that was only just
                # produced (the ACT->DVE->v chain is ~1.2us deep).
                from collections import deque
                pending = deque()  # (g0, t_v) awaiting scatter

                # Input DMA routing: all rhs chunks are issued upfront from
                # the Pool queue (SWDGE) so they are never queued behind a wa
                # pool-slot wait -- if they were, the Tile scheduler's own
                # pipeline sim would see theta matmuls blocked on rhs and lock
                # a scatter-before-theta PE order that stalls ACT at runtime.
                # wa chunks stream on SP, self-paced by their pool slots.
                n_chunks = (n_batches + DMA_B - 1) // DMA_B
                rhs_all = rhsp.tile([KR, ncols], bf16, tag="rhs")
                wa_tiles = []

                def emit_rhs_dma(k):
                    col0 = k * DMA_B * NB
                    W2 = min(DMA_B, n_batches - k * DMA_B) * NB
                    # chunk 0 from SP (fast HWDGE, ahead of wa0 on the DMA
                    # engines -- it gates the first theta); the rest from
                    # Pool/SWDGE so they never sit behind a wa slot wait
                    eng = nc.sync if k == 0 else nc.gpsimd
                    eng.dma_start(
                        rhs_all[:, col0:col0 + W2], d_rhs[:, col0:col0 + W2])

                def emit_wa_dma(k):
                    col0 = k * DMA_B * NB
                    W2 = min(DMA_B, n_batches - k * DMA_B) * NB
                    t = wap.tile([128, DMA_B * NB], bf16, tag="wa")
                    nc.sync.dma_start(t[:, :W2], d_wa[:, col0:col0 + W2])
                    wa_tiles.append(t)

                for k in range(n_chunks):
                    emit_rhs_dma(k)
                for k in range(n_chunks):
                    emit_wa_dma(k)
                    if k == 0:
                        nc.sync.dma_start(iden[:], d_iden[:])

                for b2 in range(0, n_batches, DMA_B):
                    nb2 = min(DMA_B, n_batches - b2)
                    t_wa2 = wa_tiles[b2 // DMA_B]

                    for b in range(b2, b2 + nb2):
                        part = (b - b2) * NB
                        g0 = b * BATCH
                        t_rhs = rhs_all[:, b * NB:(b + 1) * NB]

                        # the last batch runs at GPB-grain granularity so the
                        # end-of-program ACT->mul->scatter chain is 1/3 the
                        # depth; fully-dummy sub-units are skipped outright
                        lastb = b == n_batches - 1
                        nsub = -(-(GPC - g0) // GPB) if lastb else 3
                        nsub = max(1, min(3, nsub))

                        th = thp.tile([128, 3 * 512], f32, tag="th")
                        # priority-bias the theta matmuls ahead of the
                        # previous batch's scatter matmuls so PE's in-order
                        # dispatch never blocks them behind a v-wait.
                        with tc.high_priority(offset=16):
                            for m in range(nsub):
                                sl = slice(
                                    m * GPB * GCOLS, (m + 1) * GPB * GCOLS)
                                nc.tensor.matmul(
                                    th[:, m * 512: m * 512 + GPB * GCOLS],
                                    lhs_t[:],
                                    t_rhs[:, sl],
                                    start=True, stop=True,
                                )
                        t_s = sp.tile([128, NB], bf16, tag="s")
                        t_v = vp.tile([128, NB], bf16, tag="v")
                        if not lastb:
                            th3 = th[:].rearrange(
                                "p (b x) -> p b x", b=3)[:, :, :GPB * GCOLS]
                            s3 = t_s[:].rearrange("p (b x) -> p b x", b=3)
                            nc.scalar.activation(s3, th3, AF.Sin, scale=TWO_PI)
                            nc.vector.tensor_mul(
                                t_v[:], t_s[:], t_wa2[:, part:part + NB])
                            pending.append((g0, t_v, BATCH, 0))
                            if len(pending) > SCATTER_LAG:
                                emit_scatter(*pending.popleft())
                        else:
                            U = GPB * GCOLS
                            for m in range(nsub):
                                nc.scalar.activation(
                                    t_s[:, m * U:(m + 1) * U],
                                    th[:, m * 512: m * 512 + U],
                                    AF.Sin, scale=TWO_PI)
                                nc.vector.tensor_mul(
                                    t_v[:, m * U:(m + 1) * U],
                                    t_s[:, m * U:(m + 1) * U],
                                    t_wa2[:, part + m * U:part + (m + 1) * U])
                                pending.append((g0 + m * GPB, t_v, GPB, m * U))
                                if len(pending) > SCATTER_LAG:
                                    emit_scatter(*pending.popleft())
                while pending:
                    emit_scatter(*pending.popleft())
                # final (small) strip: evacuate and flush the remainder
                base, w = cur[0], cur[3] - cur[0]
                nc.vector.tensor_add(
                    acc[:, base:base + w], cur_tile[:, :w],
                    acc[:, base:base + w])
                nc.sync.dma_start(
                    flush_dst[:, flushed:ACC_COLS],
                    acc[:, flushed:ACC_COLS])
                del wa_tiles[:]

        if single_core:
            d_full = nc.dram_tensor(
                "full", [128, ACC_COLS], f32, kind="ExternalOutput").ap()
            emit_core_body(0)
        else:
            pid = nc.partition_id()
            for core in range(N_CORES):
                with tc.If(pid == core):
                    emit_core_body(core)

            # ---- shared epilog: reduce, normalize, output ----
            b_rs = dram.tile([16, ACC_COLS], f32)
            nc.gpsimd.collective_compute(
                "ReduceScatter",
                mybir.AluOpType.add,
                replica_groups=[list(range(N_CORES))],
                ins=[b_in[:].opt()],
                outs=[b_rs[:].opt()],
            )
            red = outer.tile([128, 512], f32)
            nc.sync.dma_start(
                red[:],
                b_rs[:].rearrange("a b -> (a b)").rearrange(
                    "(p c) -> p c", p=128))

            # sum of squares of the local chunk
            scr = outer.tile([128, 512], f32)
            sqcol = outer.tile([128, 1], f32)
            nc.scalar.activation(scr[:], red[:], AF.Square, accum_out=sqcol[:])
            ones = outer.tile([128, 128], f32)
            nc.vector.memset(ones[:], 1.0)
            psq = octx.enter_context(tc.tile_pool(name="psq", bufs=1, space="PSUM"))
            ps_s = psq.tile([1, 128], f32)
            nc.tensor.matmul(ps_s[:], sqcol[:], ones[:], start=True, stop=True)
            ssq = outer.tile([1, 128], f32)
            nc.vector.tensor_copy(ssq[:], ps_s[:])

            b_s1 = dram.tile([1, 128], f32)
            b_s2 = dram.tile([1, 128], f32)
            nc.sync.dma_start(b_s1[:], ssq[:])
            nc.gpsimd.collective_compute(
                "AllReduce",
                mybir.AluOpType.add,
                replica_groups=[list(range(N_CORES))],
                ins=[b_s1[:].opt()],
                outs=[b_s2[:].opt()],
            )
            gsq = outer.tile([1, 1], f32)
            nc.sync.dma_start(gsq[:], b_s2[:, 0:1])

            # rscale = rsqrt(gsq) with one Newton refinement
            nrm = outer.tile([1, 1], f32)
            nc.scalar.activation(nrm[:], gsq[:], AF.Sqrt)
            z0 = outer.tile([1, 1], f32)
            nc.vector.reciprocal(z0[:], nrm[:])
            z2 = outer.tile([1, 1], f32)
            nc.vector.tensor_mul(z2[:], z0[:], z0[:])
            t2 = outer.tile([1, 1], f32)
            nc.vector.tensor_mul(t2[:], z2[:], gsq[:])
            t3 = outer.tile([1, 1], f32)
            nc.vector.tensor_scalar(t3[:], t2[:], -0.5, 1.5, ALU.mult, ALU.add)
            z1 = outer.tile([1, 1], f32)
            nc.vector.tensor_mul(z1[:], z0[:], t3[:])

            # broadcast to 128 partitions via DRAM bounce
            b_z = dram.tile([1, 1], f32)
            nc.sync.dma_start(b_z[:], z1[:])
            zb = outer.tile([128, 1], f32)
            bz_ap = b_z[:]
            bcast = bass.AP(tensor=bz_ap.tensor, offset=bz_ap.offset,
                            ap=[[0, 128], [1, 1]])
            nc.sync.dma_start(zb[:], bcast)

            outt = outer.tile([128, 512], f32)
            nc.vector.tensor_scalar(outt[:], red[:], zb[:], None, ALU.mult)
            nc.sync.dma_start(
                d_out.rearrange("(p c) -> p c", p=128), outt[:])

    nc.compile()
    return nc


def estimate_hw_time_ns(theta_density, theta_slope, f0_freqs_hz, onsets):
    """Cost-model (TimelineSim) estimate of one core's execution, ns.

    Single-core variant: core 0's synthesis+scatter+evac plus the 2MB
    accumulator DMA-out (standing in for the ReduceScatter contribution).
    """
    from concourse.timeline_sim import TimelineSim

    cores, meta = _host_prep(theta_density, theta_slope, f0_freqs_hz, onsets)
    nc = _build_program(cores, meta, single_core=True)
    ts = TimelineSim(nc)
    ts.simulate()
    return float(ts.time)


def kernel(theta_density, theta_slope, f0_freqs_hz, onsets):
    import ml_dtypes
    from concourse import bass_utils

    cores, meta = _host_prep(theta_density, theta_slope, f0_freqs_hz, onsets)
    nc = _build_program(cores, meta)

    iden = np.eye(128, dtype=np.float32).astype(ml_dtypes.bfloat16)
    in_maps = []
    for c in range(N_CORES):
        info = cores[c]
        in_maps.append(
            dict(
                lhs=meta["lhs"],
                rhs=info["rhs"],
                wa=info["wa"],
                iden=iden,
            )
        )
    res = bass_utils.run_bass_kernel_spmd(
        nc, in_maps, core_ids=list(range(N_CORES)))

    X = np.zeros((ACC_COLS, 128), dtype=np.float32)
    for c in range(N_CORES):
        chunk = res.results[c]["out"].reshape(16, ACC_COLS)
        X[:, 16 * c:16 * (c + 1)] = chunk.T
    return X.reshape(-1).astype(np.float32)


if __name__ == "__main__":
    rng = np.random.default_rng(0)
    out = kernel(
        np.float32(0.5), np.float32(0.3),
        np.exp(rng.uniform(np.log(F0_MIN), np.log(F0_MAX), N_GRAINS)).astype(np.float32),
        rng.integers(0, N_SAMPLES - GRAIN_N, N_GRAINS).astype(np.int32),
    )
    print(out.shape, out[:8], np.linalg.norm(out))



# revision 40
# speedup vs baseline: 1.2989x; 1.0355x over previous
"""ChirpTextureSynth Trainium2 kernel.

Synthesizes 4096 windowed chirp grains (16384 samples each), scatter-adds
them at per-grain onsets into a 524288-sample signal, L2-normalizes.

Strategy (8 NeuronCores, data-parallel over grains, 512 grains/core):
 - Output accumulator layout: sample s -> (partition p = s % 128, col = s // 128).
   A grain at onset o occupies cols [o//128, o//128 + 129) on all partitions
   (onsets never wrap: o < N_SAMPLES - GRAIN_N).
 - Sine argument in CYCLES: theta(p,c) = f0*phase(t), t = i/SR - D/2,
   i = 128*c + p - (o % 128). theta is low-rank separable in (p, c):
   exp branch  : theta = a*E(p)*F(c) + b,  E(p)=exp(g*p/SR)
   taylor branch (|g| < 0.7): theta = sum_j coeff_j(c) * p^j, j=0..4
 - Range reduction is folded INTO the theta matmul: S piecewise-constant
   partition-segment hint rows (indicator lhsT rows x integer bf16 rhs rows)
   subtract round(theta) per segment, leaving |u| <= ~0.58.  The ACT Sin
   spline (scale=2pi) is accurate to ~3e-4 out to |u|=0.58, so ONE K<=128
   matmul per 3 grains produces ready-to-sin u tiles in PSUM (f32).
   Fallback for extreme chirp slopes (segment residual too big): per-element
   int8 hint plane applied via identity matmul / DVE, as before.
 - ACT Sin(scale=2pi) evaluates sin(2*pi*u) -> bf16 SBUF.
 - Window*amp (bf16 host table WA) applied on DVE: v = s * WA (2x bf16).
 - Scatter: per-grain matmul with identity weights accumulates v into a
   PSUM "strip" bank (512 cols); strips follow onset-sorted grains; DVE
   evacuates each strip into the SBUF accumulator.  Columns left of the
   next strip's base are final and are streamed out to DRAM eagerly.
 - Per-core instruction streams differ (grain offsets are immediates), so the
   program has 8 tc.If(partition_id == c) branches; inputs differ per core.
 - Reduction: ReduceScatter (128x4096 f32) + scalar AllReduce of sum-of-
   squares; each core normalizes and outputs its 1/8 chunk; host reassembles.
"""

import math
from contextlib import ExitStack

import numpy as np

SR = 44100.0
N_SAMPLES = 524288
N_GRAINS = 4096
GRAIN_N = 16384
F0_MIN = 32.7
F0_MAX = 523.25
Q = 12
HOP_LEN = 256
GRAIN_DUR_S = GRAIN_N / SR
N_CORES = 8
GPC = N_GRAINS // N_CORES  # grains per core (512)

ACC_COLS = N_SAMPLES // 128        # 4096
ACC_PAD_COLS = ACC_COLS + 384      # strip overhang room
GCOLS = 109                        # cols per grain tile; (129-GCOLS)/2 cols
                                   # trimmed per edge where the Hann window is
                                   # tiny (rel-L2 cost ~10*f^2.5, f=trim frac)
BATCH = 12                         # grains per compute batch (3 psum banks)
GPB = 4                            # grains per theta-matmul (508 cols <= 512)
TAYLOR_CUT = 0.7                   # |gamma| below which the poly branch is used
STRIP_COLS = 512
SEG_MARGIN = 0.65                  # max |u| the Sin spline tolerates (~5e-3)
DMA_B = 2                          # batches per input DMA chunk
DMA_PRE = 5                        # chunks interleaved rhs/wa at the start
FLUSH_MIN = 384                    # min final cols before an output flush
SCATTER_LAG = 4                    # batches the scatter trails the sin chain


def _host_prep(theta_density, theta_slope, f0_freqs_hz, onsets):
    """All host-side precompute. Returns per-core input arrays + metadata."""
    td = float(np.float32(theta_density))
    ts = float(np.float32(theta_slope))
    f0 = np.asarray(f0_freqs_hz, dtype=np.float64)
    on = np.asarray(onsets, dtype=np.int64)

    # per-grain amplitudes (matches reference, f64 is fine vs f32 ref)
    gi = np.arange(N_GRAINS, dtype=np.float64)
    offset = 0.25 * td + 0.75 * td * td
    sig_op = (1.0 - td) * N_GRAINS * (gi / N_GRAINS - offset)
    amps = 0.5 * (1.0 - np.tanh(sig_op))  # = 1 - sigmoid(2*sig_op), stable
    amps = amps / amps.max()
    A = amps / np.sqrt(f0)

    typical_slope = SR / (Q * HOP_LEN)
    gamma = math.tan(ts * math.pi / 2.0) * typical_slope / 4.0

    use_exp = abs(gamma) >= TAYLOR_CUT

    # padded grain count per core -> multiple of BATCH
    gpc_pad = ((GPC + BATCH - 1) // BATCH) * BATCH   # 513
    n_batches = gpc_pad // BATCH

    import ml_dtypes

    def bsplit(x, n):
        """Split f64 array into n bf16 parts summing to ~x (24 bits for n=3)."""
        parts = []
        rem = np.array(x, dtype=np.float64, copy=True)
        for _ in range(n):
            h = rem.astype(ml_dtypes.bfloat16)
            parts.append(h)
            rem = rem - h.astype(np.float64)
        return parts

    pvec = np.arange(128, dtype=np.float64)
    # base lhsT rows [KB, 128] in bf16; theta matmul runs at bf16 rate.
    # exp branch rows:    [Eh,Eh,Eh,Em,Em,El, 1, 1, 1]
    #   paired rhs rows:  [Rh,Rm,Rl,Rh,Rm,Rh, bh,bm,bl]
    # taylor branch rows: [1,1,1, p,p,p, p2h,p2h,p2l, p3,p3, p4]
    #   paired rhs rows:  [c0h,c0m,c0l, c1h,c1m,c1l, c2h,c2l,c2h, c3h,c3l, c4h]
    KB = 9 if use_exp else 12
    lhs_base = np.zeros((KB, 128), dtype=np.float64)
    if use_exp:
        E = np.exp(gamma * pvec / SR)
        Eh, Em, El = bsplit(E, 3)
        for i, v in enumerate([Eh, Eh, Eh, Em, Em, El]):
            lhs_base[i] = np.asarray(v, dtype=np.float64)
        lhs_base[6] = lhs_base[7] = lhs_base[8] = 1.0
    else:
        lhs_base[0] = lhs_base[1] = lhs_base[2] = 1.0
        lhs_base[3] = lhs_base[4] = lhs_base[5] = pvec
        p2h, p2l = bsplit(pvec ** 2, 2)
        lhs_base[6] = lhs_base[7] = np.asarray(p2h, np.float64)
        lhs_base[8] = np.asarray(p2l, np.float64)
        p3h = bsplit(pvec ** 3, 1)[0]
        lhs_base[9] = lhs_base[10] = np.asarray(p3h, np.float64)
        lhs_base[11] = np.asarray(bsplit(pvec ** 4, 1)[0], np.float64)
    lhs_base_bf = lhs_base.astype(ml_dtypes.bfloat16)
    lhs_base_f64 = lhs_base_bf.astype(np.float64)

    # tile col c covers samples i = 128*(c+TRIM_L) + p - r (grain-local),
    # i.e. global cols q+c with q = onset//128 + TRIM_L: the (129-GCOLS)
    # dropped edge cols carry only the Hann window's faint tails.
    TRIM_L = (129 - GCOLS) // 2
    cvec = np.arange(GCOLS, dtype=np.float64) + TRIM_L
    ncols = gpc_pad * GCOLS

    fact = [1, 1, 2, 6, 24, 120]

    cores = []
    seg_S = 8  # shared across cores (program structure is per-core anyway)
    for c in range(N_CORES):
        gsel = np.arange(c * GPC, (c + 1) * GPC)
        q = on[gsel] // 128 + TRIM_L
        order = np.argsort(q, kind="stable")
        gsel = gsel[order]
        q = q[order]
        r = on[gsel] % 128

        # strip assignment (greedy, span <= STRIP_COLS, no coverage gaps)
        strips = []  # list of [base, first_idx, last_idx, covered_end]
        base = None
        for k in range(GPC):
            qk = int(q[k])
            if (base is None or qk + GCOLS > base + STRIP_COLS
                    or qk > strips[-1][3]):
                base = qk
                strips.append([base, k, k, qk + GCOLS])
            else:
                strips[-1][2] = k
                strips[-1][3] = max(strips[-1][3], qk + GCOLS)
        # split the trailing strip so the final strip holds only the last few
        # grains: the (large) penultimate strip then evacuates and flushes
        # during the drain, leaving a short end-of-program chain
        split_at = GPC - GPB
        if strips[-1][1] < split_at:
            old = strips[-1]
            cov = int(max(q[old[1]:split_at])) + GCOLS
            strips[-1] = [old[0], old[1], split_at - 1, cov]
            strips.append([int(q[split_at]), split_at, GPC - 1, old[3]])
        # dummy (pad) grains have zero amplitude; their scatters are skipped
        # entirely (emit_scatter guards on g >= GPC), so strips only cover
        # real grains
        n_dummy = gpc_pad - GPC
        q_dummy = strips[-1][0]

        f0c = f0[gsel]
        Ac = A[gsel]

        # ideal theta model at p=0 (for the per-column base), [GPC, 129]
        # beta[g, c] = (128*c - r_g)/SR - D/2   (t at p=0)
        beta = (128.0 * cvec[None, :] - r[:, None]) / SR - GRAIN_DUR_S / 2.0
        if use_exp:
            a_g = f0c / gamma
            R_ideal = (a_g[:, None]) * np.exp(gamma * beta)
            const_ideal = np.broadcast_to(-a_g[:, None], beta.shape)
            theta0 = R_ideal + const_ideal  # theta at p=0
        else:
            coeff = np.zeros((5, GPC, GCOLS), dtype=np.float64)  # j = 0..4
            for k in range(1, 6):
                gk = gamma ** (k - 1) / fact[k]
                for j in range(0, min(k, 4) + 1):
                    binom = math.comb(k, j)
                    coeff[j] += gk * binom * beta ** (k - j) * SR ** (-j)
            coeff *= f0c[None, :, None]
            theta0 = coeff[0]

        base_c = np.round(theta0)  # folded into the const row -> |theta'| small

        # build bf16-split rhs base rows [KB, ncols]
        rhs64 = np.zeros((KB, ncols), dtype=np.float64)

        def put(row, arr):
            rhs64[row, : GPC * GCOLS] = np.asarray(arr, np.float64).reshape(-1)

        if use_exp:
            Rh, Rm, Rl = bsplit(R_ideal, 3)
            bh, bm, bl = bsplit(const_ideal - base_c, 3)
            for i, v in enumerate([Rh, Rm, Rl, Rh, Rm, Rh, bh, bm, bl]):
                put(i, v)
        else:
            c0h, c0m, c0l = bsplit(coeff[0] - base_c, 3)
            c1h, c1m, c1l = bsplit(coeff[1], 3)
            c2h, c2l = bsplit(coeff[2], 2)
            c3h, c3l = bsplit(coeff[3], 2)
            c4h = bsplit(coeff[4], 1)[0]
            for i, v in enumerate([c0h, c0m, c0l, c1h, c1m, c1l,
                                   c2h, c2l, c2h, c3h, c3l, c4h]):
                put(i, v)
        rhs_base = rhs64.astype(ml_dtypes.bfloat16)

        # device-model theta (f64 sim of the bf16 matmul), [128, ncols]
        theta = lhs_base_f64.T @ rhs_base.astype(np.float64)

        # segment hints: S partition segments, integer hint per (segment, col)
        S = seg_S
        while True:
            t3 = theta.reshape(S, 128 // S, ncols)
            mid = 0.5 * (t3.max(axis=1) + t3.min(axis=1))
            rseg = np.round(mid)  # [S, ncols] integers
            resid = np.abs(t3 - rseg[:, None, :]).max()
            if resid <= SEG_MARGIN or S >= 64:
                break
            S *= 2
        seg_S = max(seg_S, S)

        cores.append(
            dict(
                rhs_base=rhs_base,
                theta=theta,
                r=r,
                q=np.concatenate([q, np.full(n_dummy, q_dummy, dtype=np.int64)]),
                strips=strips,
                Ac=Ac,
            )
        )

    # second pass: uniform S across cores; build final lhs/rhs (+wa) arrays
    S = seg_S
    KR = KB + S
    seg = 128 // S
    lhs = np.zeros((KR, 128), dtype=np.float64)
    lhs[:KB] = lhs_base_f64
    for k in range(S):
        lhs[KB + k, k * seg:(k + 1) * seg] = 1.0
    lhs_bf = lhs.astype(ml_dtypes.bfloat16)

    resid_max = 0.0
    fallback = False
    pvec_i = np.arange(128)
    for c in range(N_CORES):
        info = cores[c]
        theta = info.pop("theta")
        t3 = theta.reshape(S, seg, ncols)
        mid = 0.5 * (t3.max(axis=1) + t3.min(axis=1))
        rseg = np.round(mid)  # [S, ncols]
        resid = np.abs(t3 - rseg[:, None, :]).max()
        resid_max = max(resid_max, resid)
        assert np.abs(rseg).max() < 250, "segment hint exceeds bf16-exact range"
        rhs = np.zeros((KR, ncols), dtype=np.float64)
        rhs[:KB] = info.pop("rhs_base").astype(np.float64)
        rhs[KB:] = -rseg
        info["rhs"] = rhs.astype(ml_dtypes.bfloat16)

        # WA table: A_g * W(i), i = 128*c + p - r_g.  The sample-range mask is
        # dropped: outside [0, GRAIN_N) the Hann window value is ~(pi*i/N)^2
        # <= 6e-4, negligible vs the 2e-2 error budget.
        r = info.pop("r")
        Ac = info.pop("Ac")
        i_idx = (128 * cvec[None, None, :] + pvec_i[None, :, None]
                 - r[:, None, None])  # [GPC, 128, 129]
        W = np.sin(np.pi * i_idx / GRAIN_N) ** 2
        WA = (W * Ac[:, None, None]).transpose(1, 0, 2).reshape(128, GPC * GCOLS)
        wa_full = np.zeros((128, ncols), dtype=np.float64)
        wa_full[:, : GPC * GCOLS] = WA
        info["wa"] = wa_full.astype(ml_dtypes.bfloat16)

    if resid_max > SEG_MARGIN:
        fallback = True  # extreme chirp slope: per-element hints needed

    meta = dict(
        lhs=lhs_bf,
        gpc_pad=gpc_pad,
        n_batches=n_batches,
        use_exp=use_exp,
        gamma=gamma,
        ncols=ncols,
        KR=KR,
        fallback=fallback,
        resid=resid_max,
    )
    return cores, meta


def _build_program(cores, meta, single_core=False):
    import concourse.bacc as bacc
    import concourse.bass as bass
    import concourse.tile as tile
    import concourse.mybir as mybir
    from concourse import bass_utils  # noqa: F401

    ncols = meta["ncols"]
    n_batches = meta["n_batches"]
    KR = meta["KR"]

    nc = bacc.Bacc("TRN2", target_bir_lowering=False, debug=False,
                   num_devices=1 if single_core else N_CORES)
    f32 = mybir.dt.float32
    bf16 = mybir.dt.bfloat16

    d_lhs = nc.dram_tensor("lhs", [KR, 128], bf16, kind="ExternalInput").ap()
    d_rhs = nc.dram_tensor("rhs", [KR, ncols], bf16, kind="ExternalInput").ap()
    d_wa = nc.dram_tensor("wa", [128, ncols], bf16, kind="ExternalInput").ap()
    d_iden = nc.dram_tensor("iden", [128, 128], bf16, kind="ExternalInput").ap()
    d_out = nc.dram_tensor("out", [65536], f32, kind="ExternalOutput").ap()

    AF = mybir.ActivationFunctionType
    ALU = mybir.AluOpType
    TWO_PI = float(2.0 * np.pi)

    with tile.TileContext(nc) as tc, ExitStack() as octx:
        outer = octx.enter_context(tc.tile_pool(name="outer", bufs=1))
        # zero tile fueling PE warm-up matmuls (see emit_core_body)
        wz = outer.tile([128, 128], bf16)
        nc.vector.memset(wz[:], 0.0)
        acc = outer.tile([128, ACC_PAD_COLS], f32)
        # memset on DVE (idle at startup): on Pool it would sit ahead of the
        # rhs SWDGE descriptor generations and delay the first theta by ~4us
        nc.vector.memset(acc[:], 0.0)
        lhs_t = outer.tile([KR, 128], bf16)
        # lhs goes out on Pool's DGE so rhs chunk 0 is SP's first issue --
        # the two tiny transfers then overlap instead of pacing 625ns apart
        nc.gpsimd.dma_start(lhs_t[:], d_lhs[:])
        iden = outer.tile([128, 128], bf16)
        # iden's DMA is issued inside the core body after the first input
        # chunk: it is first read only at the first scatter (~10us in) and
        # must not delay rhs chunk 0 on the DMA engines.

        if not single_core:
            dram = octx.enter_context(
                tc.tile_pool(name="dram", bufs=1, space="DRAM"))
            b_in = dram.tile([128, ACC_COLS], f32)

        def emit_core_body(core):
            """Returns nothing; flushes final acc columns eagerly into the
            reduce input (SPMD) or the output (single-core estimate)."""
            info = cores[core]
            q = info["q"]
            strips = info["strips"]

            if single_core:
                flush_dst = d_full
            else:
                flush_dst = b_in[:]

            with ExitStack() as ctx:
                rhsp = ctx.enter_context(
                    tc.tile_pool(name=f"rhs{core}", bufs=1))
                wap = ctx.enter_context(tc.tile_pool(name=f"wap{core}", bufs=5))
                sp = ctx.enter_context(tc.tile_pool(name=f"sp{core}", bufs=3))
                vp = ctx.enter_context(
                    tc.tile_pool(name=f"vp{core}", bufs=SCATTER_LAG + 2))
                thp = ctx.enter_context(
                    tc.tile_pool(name=f"th{core}", bufs=2, space="PSUM"))
                stp = ctx.enter_context(
                    tc.tile_pool(name=f"st{core}", bufs=2, space="PSUM"))

                # PE warm-up: ~2us of throwaway matmuls on zeros so the PE
                # p-state ramp is past its low tier when rhs chunk 0 lands
                wt = stp.tile([128, STRIP_COLS], f32, tag="strip")
                for _ in range(18):
                    nc.tensor.matmul(
                        wt[:, :128], wz[:], wz[:], start=True, stop=True)

                # strip state machine across batches
                strip_iter = iter(strips)
                cur = next(strip_iter)
                cur_tile = None
                flushed = 0  # acc cols already written out
                NB = BATCH * GCOLS  # 1161

                def flush_to(limit, force=False):
                    """DMA-out final acc cols [flushed, limit).  Issued from
                    the otherwise-idle Pool queue: a flush's sem-wait on
                    pending strip evacs must not delay the SP-issued input
                    DMAs."""
                    nonlocal flushed
                    lim = min(limit, ACC_COLS)
                    if lim - flushed >= FLUSH_MIN or (force and lim > flushed):
                        # the forced (drain-time) flush takes SP's HWDGE: SP
                        # is idle then and its issue path is ~1us faster than
                        # Pool's SWDGE generation
                        eng = nc.sync if force else nc.gpsimd
                        eng.dma_start(
                            flush_dst[:, flushed:lim], acc[:, flushed:lim])
                        flushed = lim

                def emit_scatter(g0, t_v, ng=BATCH, voff=0):
                    nonlocal cur, cur_tile
                    for j in range(ng):
                        g = g0 + j
                        if g >= GPC:  # zero-amplitude pad grain
                            continue
                        # open new strip?
                        if g > cur[2]:
                            # evacuate finished strip (covered span)
                            w = cur[3] - cur[0]
                            nc.vector.tensor_add(
                                acc[:, cur[0]:cur[0] + w],
                                cur_tile[:, :w],
                                acc[:, cur[0]:cur[0] + w],
                            )
                            cur = next(strip_iter)
                            cur_tile = None
                            # at the last strip, flush everything below its
                            # base now so the tail flush only covers its span
                            flush_to(cur[0], force=cur is strips[-1])
                        first = cur_tile is None
                        if first:
                            cur_tile = stp.tile(
                                [128, STRIP_COLS], f32, tag="strip")
                        off = int(q[g]) - cur[0]
                        last = g == cur[2]
                        nc.tensor.matmul(
                            cur_tile[:, off:off + GCOLS],
                            iden[:],
                            t_v[:, voff + j * GCOLS:voff + (j + 1) * GCOLS],
                            start=first, stop=last,
                        )

                # software pipeline: scatter runs SCATTER_LAG batches behind
                # the theta->sin->window chain so PE's in-order queue never
                # blocks upcoming theta matmuls on a v that was only just
                # produced (the ACT->DVE->v chain is ~1.2us deep).
                from collections import deque
                pending = deque()  # (g0, t_v) awaiting scatter

                # Input DMA routing: all rhs chunks are issued upfront from
                # the Pool queue (SWDGE) so they are never queued behind a wa
                # pool-slot wait -- if they were, the Tile scheduler's own
                # pipeline sim would see theta matmuls blocked on rhs and lock
                # a scatter-before-theta PE order that stalls ACT at runtime.
                # wa chunks stream on SP, self-paced by their pool slots.
                n_chunks = (n_batches + DMA_B - 1) // DMA_B
                rhs_all = rhsp.tile([KR, ncols], bf16, tag="rhs")
                wa_tiles = []

                def emit_rhs_dma(k):
                    col0 = k * DMA_B * NB
                    W2 = min(DMA_B, n_batches - k * DMA_B) * NB
                    # chunk 0 from SP (fast HWDGE, ahead of wa0 on the DMA
                    # engines -- it gates the first theta); the rest from
                    # Pool/SWDGE so they never sit behind a wa slot wait
                    eng = nc.sync if k == 0 else nc.gpsimd
                    eng.dma_start(
                        rhs_all[:, col0:col0 + W2], d_rhs[:, col0:col0 + W2])

                def emit_wa_dma(k):
                    col0 = k * DMA_B * NB
                    W2 = min(DMA_B, n_batches - k * DMA_B) * NB
                    t = wap.tile([128, DMA_B * NB], bf16, tag="wa")
                    nc.sync.dma_start(t[:, :W2], d_wa[:, col0:col0 + W2])
                    wa_tiles.append(t)

                for k in range(n_chunks):
                    emit_rhs_dma(k)
                for k in range(n_chunks):
                    emit_wa_dma(k)
                    if k == 0:
                        nc.sync.dma_start(iden[:], d_iden[:])

                for b2 in range(0, n_batches, DMA_B):
                    nb2 = min(DMA_B, n_batches - b2)
                    t_wa2 = wa_tiles[b2 // DMA_B]

                    for b in range(b2, b2 + nb2):
                        part = (b - b2) * NB
                        g0 = b * BATCH
                        t_rhs = rhs_all[:, b * NB:(b + 1) * NB]

                        # the last batch runs at GPB-grain granularity so the
                        # end-of-program ACT->mul->scatter chain is 1/3 the
                        # depth; fully-dummy sub-units are skipped outright
                        lastb = b == n_batches - 1
                        nsub = -(-(GPC - g0) // GPB) if lastb else 3
                        nsub = max(1, min(3, nsub))

                        th = thp.tile([128, 3 * 512], f32, tag="th")
                        # priority-bias the theta matmuls ahead of the
                        # previous batch's scatter matmuls so PE's in-order
                        # dispatch never blocks them behind a v-wait.
                        with tc.high_priority(offset=16):
                            for m in range(nsub):
                                sl = slice(
                                    m * GPB * GCOLS, (m + 1) * GPB * GCOLS)
                                nc.tensor.matmul(
                                    th[:, m * 512: m * 512 + GPB * GCOLS],
                                    lhs_t[:],
                                    t_rhs[:, sl],
                                    start=True, stop=True,
                                )
                        t_s = sp.tile([128, NB], bf16, tag="s")
                        t_v = vp.tile([128, NB], bf16, tag="v")
                        if not lastb:
                            th3 = th[:].rearrange(
                                "p (b x) -> p b x", b=3)[:, :, :GPB * GCOLS]
                            s3 = t_s[:].rearrange("p (b x) -> p b x", b=3)
                            nc.scalar.activation(s3, th3, AF.Sin, scale=TWO_PI)
                            nc.vector.tensor_mul(
                                t_v[:], t_s[:], t_wa2[:, part:part + NB])
                            pending.append((g0, t_v, BATCH, 0))
                            if len(pending) > SCATTER_LAG:
                                emit_scatter(*pending.popleft())
                        else:
                            U = GPB * GCOLS
                            for m in range(nsub):
                                nc.scalar.activation(
                                    t_s[:, m * U:(m + 1) * U],
                                    th[:, m * 512: m * 512 + U],
                                    AF.Sin, scale=TWO_PI)
                                nc.vector.tensor_mul(
                                    t_v[:, m * U:(m + 1) * U],
                                    t_s[:, m * U:(m + 1) * U],
                                    t_wa2[:, part + m * U:part + (m + 1) * U])
                                pending.append((g0 + m * GPB, t_v, GPB, m * U))
                                if len(pending) > SCATTER_LAG:
                                    emit_scatter(*pending.popleft())
                while pending:
                    emit_scatter(*pending.popleft())
                # final (small) strip: evacuate and flush the remainder
                base, w = cur[0], cur[3] - cur[0]
                nc.vector.tensor_add(
                    acc[:, base:base + w], cur_tile[:, :w],
                    acc[:, base:base + w])
                nc.sync.dma_start(
                    flush_dst[:, flushed:ACC_COLS],
                    acc[:, flushed:ACC_COLS])
                del wa_tiles[:]

        if single_core:
            d_full = nc.dram_tensor(
                "full", [128, ACC_COLS], f32, kind="ExternalOutput").ap()
            emit_core_body(0)
        else:
            pid = nc.partition_id()
            for core in range(N_CORES):
                with tc.If(pid == core):
                    emit_core_body(core)

            # ---- shared epilog: reduce, normalize, output ----
            b_rs = dram.tile([16, ACC_COLS], f32)
            nc.gpsimd.collective_compute(
                "ReduceScatter",
                mybir.AluOpType.add,
                replica_groups=[list(range(N_CORES))],
                ins=[b_in[:].opt()],
                outs=[b_rs[:].opt()],
            )
            red = outer.tile([128, 512], f32)
            nc.sync.dma_start(
                red[:],
                b_rs[:].rearrange("a b -> (a b)").rearrange(
                    "(p c) -> p c", p=128))

            # sum of squares of the local chunk
            scr = outer.tile([128, 512], f32)
            sqcol = outer.tile([128, 1], f32)
            nc.scalar.activation(scr[:], red[:], AF.Square, accum_out=sqcol[:])
            ones = outer.tile([128, 128], f32)
            nc.vector.memset(ones[:], 1.0)
            psq = octx.enter_context(tc.tile_pool(name="psq", bufs=1, space="PSUM"))
            ps_s = psq.tile([1, 128], f32)
            nc.tensor.matmul(ps_s[:], sqcol[:], ones[:], start=True, stop=True)
            ssq = outer.tile([1, 128], f32)
            nc.vector.tensor_copy(ssq[:], ps_s[:])

            b_s1 = dram.tile([1, 128], f32)
            b_s2 = dram.tile([1, 128], f32)
            nc.sync.dma_start(b_s1[:], ssq[:])
            nc.gpsimd.collective_compute(
                "AllReduce",
                mybir.AluOpType.add,
                replica_groups=[list(range(N_CORES))],
                ins=[b_s1[:].opt()],
                outs=[b_s2[:].opt()],
            )
            gsq = outer.tile([1, 1], f32)
            nc.sync.dma_start(gsq[:], b_s2[:, 0:1])

            # rscale = rsqrt(gsq) with one Newton refinement
            nrm = outer.tile([1, 1], f32)
            nc.scalar.activation(nrm[:], gsq[:], AF.Sqrt)
            z0 = outer.tile([1, 1], f32)
            nc.vector.reciprocal(z0[:], nrm[:])
            z2 = outer.tile([1, 1], f32)
            nc.vector.tensor_mul(z2[:], z0[:], z0[:])
            t2 = outer.tile([1, 1], f32)
            nc.vector.tensor_mul(t2[:], z2[:], gsq[:])
            t3 = outer.tile([1, 1], f32)
            nc.vector.tensor_scalar(t3[:], t2[:], -0.5, 1.5, ALU.mult, ALU.add)
            z1 = outer.tile([1, 1], f32)
            nc.vector.tensor_mul(z1[:], z0[:], t3[:])

            # broadcast to 128 partitions via DRAM bounce
            b_z = dram.tile([1, 1], f32)
            nc.sync.dma_start(b_z[:], z1[:])
            zb = outer.tile([128, 1], f32)
            bz_ap = b_z[:]
            bcast = bass.AP(tensor=bz_ap.tensor, offset=bz_ap.offset,
                            ap=[[0, 128], [1, 1]])
            nc.sync.dma_start(zb[:], bcast)

            outt = outer.tile([128, 512], f32)
            nc.vector.tensor_scalar(outt[:], red[:], zb[:], None, ALU.mult)
            nc.sync.dma_start(
                d_out.rearrange("(p c) -> p c", p=128), outt[:])

    nc.compile()
    return nc


def estimate_hw_time_ns(theta_density, theta_slope, f0_freqs_hz, onsets):
    """Cost-model (TimelineSim) estimate of one core's execution, ns.

    Single-core variant: core 0's synthesis+scatter+evac plus the 2MB
    accumulator DMA-out (standing in for the ReduceScatter contribution).
    """
    from concourse.timeline_sim import TimelineSim

    cores, meta = _host_prep(theta_density, theta_slope, f0_freqs_hz, onsets)
    nc = _build_program(cores, meta, single_core=True)
    ts = TimelineSim(nc)
    ts.simulate()
    return float(ts.time)


def kernel(theta_density, theta_slope, f0_freqs_hz, onsets):
    import ml_dtypes
    from concourse import bass_utils

    cores, meta = _host_prep(theta_density, theta_slope, f0_freqs_hz, onsets)
    nc = _build_program(cores, meta)

    iden = np.eye(128, dtype=np.float32).astype(ml_dtypes.bfloat16)
    in_maps = []
    for c in range(N_CORES):
        info = cores[c]
        in_maps.append(
            dict(
                lhs=meta["lhs"],
                rhs=info["rhs"],
                wa=info["wa"],
                iden=iden,
            )
        )
    res = bass_utils.run_bass_kernel_spmd(
        nc, in_maps, core_ids=list(range(N_CORES)))

    X = np.zeros((ACC_COLS, 128), dtype=np.float32)
    for c in range(N_CORES):
        chunk = res.results[c]["out"].reshape(16, ACC_COLS)
        X[:, 16 * c:16 * (c + 1)] = chunk.T
    return X.reshape(-1).astype(np.float32)


if __name__ == "__main__":
    rng = np.random.default_rng(0)
    out = kernel(
        np.float32(0.5), np.float32(0.3),
        np.exp(rng.uniform(np.log(F0_MIN), np.log(F0_MAX), N_GRAINS)).astype(np.float32),
        rng.integers(0, N_SAMPLES - GRAIN_N, N_GRAINS).astype(np.int32),
    )
    print(out.shape, out[:8], np.linalg.norm(out))



# revision 53
# speedup vs baseline: 1.2999x; 1.0008x over previous
"""ChirpTextureSynth Trainium2 kernel.

Synthesizes 4096 windowed chirp grains (16384 samples each), scatter-adds
them at per-grain onsets into a 524288-sample signal, L2-normalizes.

Strategy (8 NeuronCores, data-parallel over grains, 512 grains/core):
 - Output accumulator layout: sample s -> (partition p = s % 128, col = s // 128).
   A grain at onset o occupies cols [o//128, o//128 + 129) on all partitions
   (onsets never wrap: o < N_SAMPLES - GRAIN_N).
 - Sine argument in CYCLES: theta(p,c) = f0*phase(t), t = i/SR - D/2,
   i = 128*c + p - (o % 128). theta is low-rank separable in (p, c):
   exp branch  : theta = a*E(p)*F(c) + b,  E(p)=exp(g*p/SR)
   taylor branch (|g| < 0.7): theta = sum_j coeff_j(c) * p^j, j=0..4
 - Range reduction is folded INTO the theta matmul: S piecewise-constant
   partition-segment hint rows (indicator lhsT rows x integer bf16 rhs rows)
   subtract round(theta) per segment, leaving |u| <= ~0.58.  The ACT Sin
   spline (scale=2pi) is accurate to ~3e-4 out to |u|=0.58, so ONE K<=128
   matmul per 3 grains produces ready-to-sin u tiles in PSUM (f32).
   Fallback for extreme chirp slopes (segment residual too big): per-element
   int8 hint plane applied via identity matmul / DVE, as before.
 - ACT Sin(scale=2pi) evaluates sin(2*pi*u) -> bf16 SBUF.
 - Window*amp (bf16 host table WA) applied on DVE: v = s * WA (2x bf16).
 - Scatter: per-grain matmul with identity weights accumulates v into a
   PSUM "strip" bank (512 cols); strips follow onset-sorted grains; DVE
   evacuates each strip into the SBUF accumulator.  Columns left of the
   next strip's base are final and are streamed out to DRAM eagerly.
 - Per-core instruction streams differ (grain offsets are immediates), so the
   program has 8 tc.If(partition_id == c) branches; inputs differ per core.
 - Reduction: ReduceScatter (128x4096 f32) + scalar AllReduce of sum-of-
   squares; each core normalizes and outputs its 1/8 chunk; host reassembles.
"""

import math
from contextlib import ExitStack

import numpy as np

SR = 44100.0
N_SAMPLES = 524288
N_GRAINS = 4096
GRAIN_N = 16384
F0_MIN = 32.7
F0_MAX = 523.25
Q = 12
HOP_LEN = 256
GRAIN_DUR_S = GRAIN_N / SR
N_CORES = 8
GPC = N_GRAINS // N_CORES  # grains per core (512)

ACC_COLS = N_SAMPLES // 128        # 4096
ACC_PAD_COLS = ACC_COLS + 384      # strip overhang room
GCOLS = 109                        # cols per grain tile; (129-GCOLS)/2 cols
                                   # trimmed per edge where the Hann window is
                                   # tiny (rel-L2 cost ~10*f^2.5, f=trim frac)
BATCH = 12                         # grains per compute batch (3 psum banks)
GPB = 4                            # grains per theta-matmul (508 cols <= 512)
TAYLOR_CUT = 0.7                   # |gamma| below which the poly branch is used
STRIP_COLS = 512
SEG_MARGIN = 0.65                  # max |u| the Sin spline tolerates (~5e-3)
DMA_B = 2                          # batches per input DMA chunk
DMA_PRE = 5                        # chunks interleaved rhs/wa at the start
FLUSH_MIN = 384                    # min final cols before an output flush
SCATTER_LAG = 4                    # batches the scatter trails the sin chain


def _host_prep(theta_density, theta_slope, f0_freqs_hz, onsets):
    """All host-side precompute. Returns per-core input arrays + metadata."""
    td = float(np.float32(theta_density))
    ts = float(np.float32(theta_slope))
    f0 = np.asarray(f0_freqs_hz, dtype=np.float64)
    on = np.asarray(onsets, dtype=np.int64)

    # per-grain amplitudes (matches reference, f64 is fine vs f32 ref)
    gi = np.arange(N_GRAINS, dtype=np.float64)
    offset = 0.25 * td + 0.75 * td * td
    sig_op = (1.0 - td) * N_GRAINS * (gi / N_GRAINS - offset)
    amps = 0.5 * (1.0 - np.tanh(sig_op))  # = 1 - sigmoid(2*sig_op), stable
    amps = amps / amps.max()
    A = amps / np.sqrt(f0)

    typical_slope = SR / (Q * HOP_LEN)
    gamma = math.tan(ts * math.pi / 2.0) * typical_slope / 4.0

    use_exp = abs(gamma) >= TAYLOR_CUT

    # padded grain count per core -> multiple of BATCH
    gpc_pad = ((GPC + BATCH - 1) // BATCH) * BATCH   # 513
    n_batches = gpc_pad // BATCH

    import ml_dtypes

    def bsplit(x, n):
        """Split f64 array into n bf16 parts summing to ~x (24 bits for n=3)."""
        parts = []
        rem = np.array(x, dtype=np.float64, copy=True)
        for _ in range(n):
            h = rem.astype(ml_dtypes.bfloat16)
            parts.append(h)
            rem = rem - h.astype(np.float64)
        return parts

    pvec = np.arange(128, dtype=np.float64)
    # base lhsT rows [KB, 128] in bf16; theta matmul runs at bf16 rate.
    # exp branch rows:    [Eh,Eh,Eh,Em,Em,El, 1, 1, 1]
    #   paired rhs rows:  [Rh,Rm,Rl,Rh,Rm,Rh, bh,bm,bl]
    # taylor branch rows: [1,1,1, p,p,p, p2h,p2h,p2l, p3,p3, p4]
    #   paired rhs rows:  [c0h,c0m,c0l, c1h,c1m,c1l, c2h,c2l,c2h, c3h,c3l, c4h]
    KB = 9 if use_exp else 12
    lhs_base = np.zeros((KB, 128), dtype=np.float64)
    if use_exp:
        E = np.exp(gamma * pvec / SR)
        Eh, Em, El = bsplit(E, 3)
        for i, v in enumerate([Eh, Eh, Eh, Em, Em, El]):
            lhs_base[i] = np.asarray(v, dtype=np.float64)
        lhs_base[6] = lhs_base[7] = lhs_base[8] = 1.0
    else:
        lhs_base[0] = lhs_base[1] = lhs_base[2] = 1.0
        lhs_base[3] = lhs_base[4] = lhs_base[5] = pvec
        p2h, p2l = bsplit(pvec ** 2, 2)
        lhs_base[6] = lhs_base[7] = np.asarray(p2h, np.float64)
        lhs_base[8] = np.asarray(p2l, np.float64)
        p3h = bsplit(pvec ** 3, 1)[0]
        lhs_base[9] = lhs_base[10] = np.asarray(p3h, np.float64)
        lhs_base[11] = np.asarray(bsplit(pvec ** 4, 1)[0], np.float64)
    lhs_base_bf = lhs_base.astype(ml_dtypes.bfloat16)
    lhs_base_f64 = lhs_base_bf.astype(np.float64)

    # tile col c covers samples i = 128*(c+TRIM_L) + p - r (grain-local),
    # i.e. global cols q+c with q = onset//128 + TRIM_L: the (129-GCOLS)
    # dropped edge cols carry only the Hann window's faint tails.
    TRIM_L = (129 - GCOLS) // 2
    cvec = np.arange(GCOLS, dtype=np.float64) + TRIM_L
    ncols = gpc_pad * GCOLS

    fact = [1, 1, 2, 6, 24, 120]

    cores = []
    seg_S = 8  # shared across cores (program structure is per-core anyway)
    for c in range(N_CORES):
        gsel = np.arange(c * GPC, (c + 1) * GPC)
        q = on[gsel] // 128 + TRIM_L
        order = np.argsort(q, kind="stable")
        gsel = gsel[order]
        q = q[order]
        r = on[gsel] % 128

        # strip assignment (greedy, span <= STRIP_COLS, no coverage gaps)
        strips = []  # list of [base, first_idx, last_idx, covered_end]
        base = None
        for k in range(GPC):
            qk = int(q[k])
            if (base is None or qk + GCOLS > base + STRIP_COLS
                    or qk > strips[-1][3]):
                base = qk
                strips.append([base, k, k, qk + GCOLS])
            else:
                strips[-1][2] = k
                strips[-1][3] = max(strips[-1][3], qk + GCOLS)
        # split the trailing strip so the final strip holds only the last few
        # grains: the (large) penultimate strip then evacuates and flushes
        # during the drain, leaving a short end-of-program chain (within a
        # strip consecutive q's differ by < GCOLS, so the no-coverage-gap
        # invariant is preserved)
        split_at = GPC - GPB
        if strips[-1][1] < split_at:
            old = strips[-1]
            cov = int(max(q[old[1]:split_at])) + GCOLS
            strips[-1] = [old[0], old[1], split_at - 1, cov]
            strips.append([int(q[split_at]), split_at, GPC - 1, old[3]])
        # dummy (pad) grains have zero amplitude; their scatters are skipped
        # entirely (emit_scatter guards on g >= GPC), so strips only cover
        # real grains
        n_dummy = gpc_pad - GPC
        q_dummy = strips[-1][0]

        f0c = f0[gsel]
        Ac = A[gsel]

        # ideal theta model at p=0 (for the per-column base), [GPC, 129]
        # beta[g, c] = (128*c - r_g)/SR - D/2   (t at p=0)
        beta = (128.0 * cvec[None, :] - r[:, None]) / SR - GRAIN_DUR_S / 2.0
        if use_exp:
            a_g = f0c / gamma
            R_ideal = (a_g[:, None]) * np.exp(gamma * beta)
            const_ideal = np.broadcast_to(-a_g[:, None], beta.shape)
            theta0 = R_ideal + const_ideal  # theta at p=0
        else:
            coeff = np.zeros((5, GPC, GCOLS), dtype=np.float64)  # j = 0..4
            for k in range(1, 6):
                gk = gamma ** (k - 1) / fact[k]
                for j in range(0, min(k, 4) + 1):
                    binom = math.comb(k, j)
                    coeff[j] += gk * binom * beta ** (k - j) * SR ** (-j)
            coeff *= f0c[None, :, None]
            theta0 = coeff[0]

        base_c = np.round(theta0)  # folded into the const row -> |theta'| small

        # build bf16-split rhs base rows [KB, ncols]
        rhs64 = np.zeros((KB, ncols), dtype=np.float64)

        def put(row, arr):
            rhs64[row, : GPC * GCOLS] = np.asarray(arr, np.float64).reshape(-1)

        if use_exp:
            Rh, Rm, Rl = bsplit(R_ideal, 3)
            bh, bm, bl = bsplit(const_ideal - base_c, 3)
            for i, v in enumerate([Rh, Rm, Rl, Rh, Rm, Rh, bh, bm, bl]):
                put(i, v)
        else:
            c0h, c0m, c0l = bsplit(coeff[0] - base_c, 3)
            c1h, c1m, c1l = bsplit(coeff[1], 3)
            c2h, c2l = bsplit(coeff[2], 2)
            c3h, c3l = bsplit(coeff[3], 2)
            c4h = bsplit(coeff[4], 1)[0]
            for i, v in enumerate([c0h, c0m, c0l, c1h, c1m, c1l,
                                   c2h, c2l, c2h, c3h, c3l, c4h]):
                put(i, v)
        rhs_base = rhs64.astype(ml_dtypes.bfloat16)

        # device-model theta (f64 sim of the bf16 matmul), [128, ncols]
        theta = lhs_base_f64.T @ rhs_base.astype(np.float64)

        # segment hints: S partition segments, integer hint per (segment, col)
        S = seg_S
        while True:
            t3 = theta.reshape(S, 128 // S, ncols)
            mid = 0.5 * (t3.max(axis=1) + t3.min(axis=1))
            rseg = np.round(mid)  # [S, ncols] integers
            resid = np.abs(t3 - rseg[:, None, :]).max()
            if resid <= SEG_MARGIN or S >= 64:
                break
            S *= 2
        seg_S = max(seg_S, S)

        cores.append(
            dict(
                rhs_base=rhs_base,
                theta=theta,
                r=r,
                q=np.concatenate([q, np.full(n_dummy, q_dummy, dtype=np.int64)]),
                strips=strips,
                Ac=Ac,
            )
        )

    # second pass: uniform S across cores; build final lhs/rhs (+wa) arrays
    S = seg_S
    KR = KB + S
    seg = 128 // S
    lhs = np.zeros((KR, 128), dtype=np.float64)
    lhs[:KB] = lhs_base_f64
    for k in range(S):
        lhs[KB + k, k * seg:(k + 1) * seg] = 1.0
    lhs_bf = lhs.astype(ml_dtypes.bfloat16)

    resid_max = 0.0
    fallback = False
    pvec_i = np.arange(128)
    for c in range(N_CORES):
        info = cores[c]
        theta = info.pop("theta")
        t3 = theta.reshape(S, seg, ncols)
        mid = 0.5 * (t3.max(axis=1) + t3.min(axis=1))
        rseg = np.round(mid)  # [S, ncols]
        resid = np.abs(t3 - rseg[:, None, :]).max()
        resid_max = max(resid_max, resid)
        assert np.abs(rseg).max() < 250, "segment hint exceeds bf16-exact range"
        rhs = np.zeros((KR, ncols), dtype=np.float64)
        rhs[:KB] = info.pop("rhs_base").astype(np.float64)
        rhs[KB:] = -rseg
        info["rhs"] = rhs.astype(ml_dtypes.bfloat16)

        # WA table: A_g * W(i), i = 128*c + p - r_g.  The sample-range mask is
        # dropped: outside [0, GRAIN_N) the Hann window value is ~(pi*i/N)^2
        # <= 6e-4, negligible vs the 2e-2 error budget.
        r = info.pop("r")
        Ac = info.pop("Ac")
        i_idx = (128 * cvec[None, None, :] + pvec_i[None, :, None]
                 - r[:, None, None])  # [GPC, 128, 129]
        W = np.sin(np.pi * i_idx / GRAIN_N) ** 2
        WA = (W * Ac[:, None, None]).transpose(1, 0, 2).reshape(128, GPC * GCOLS)
        wa_full = np.zeros((128, ncols), dtype=np.float64)
        wa_full[:, : GPC * GCOLS] = WA
        info["wa"] = wa_full.astype(ml_dtypes.bfloat16)

    if resid_max > SEG_MARGIN:
        fallback = True  # extreme chirp slope: per-element hints needed

    meta = dict(
        lhs=lhs_bf,
        gpc_pad=gpc_pad,
        n_batches=n_batches,
        use_exp=use_exp,
        gamma=gamma,
        ncols=ncols,
        KR=KR,
        fallback=fallback,
        resid=resid_max,
    )
    return cores, meta


def _build_program(cores, meta, single_core=False):
    import concourse.bacc as bacc
    import concourse.bass as bass
    import concourse.tile as tile
    import concourse.mybir as mybir
    from concourse import bass_utils  # noqa: F401

    ncols = meta["ncols"]
    n_batches = meta["n_batches"]
    KR = meta["KR"]

    nc = bacc.Bacc("TRN2", target_bir_lowering=False, debug=False,
                   num_devices=1 if single_core else N_CORES)
    f32 = mybir.dt.float32
    bf16 = mybir.dt.bfloat16

    d_lhs = nc.dram_tensor("lhs", [KR, 128], bf16, kind="ExternalInput").ap()
    d_rhs = nc.dram_tensor("rhs", [KR, ncols], bf16, kind="ExternalInput").ap()
    d_wa = nc.dram_tensor("wa", [128, ncols], bf16, kind="ExternalInput").ap()
    d_iden = nc.dram_tensor("iden", [128, 128], bf16, kind="ExternalInput").ap()
    d_out = nc.dram_tensor("out", [65536], f32, kind="ExternalOutput").ap()

    AF = mybir.ActivationFunctionType
    ALU = mybir.AluOpType
    TWO_PI = float(2.0 * np.pi)

    with tile.TileContext(nc) as tc, ExitStack() as octx:
        outer = octx.enter_context(tc.tile_pool(name="outer", bufs=1))
        # zero tile fueling PE warm-up matmuls (see emit_core_body)
        wz = outer.tile([128, 128], bf16)
        nc.vector.memset(wz[:], 0.0)
        acc = outer.tile([128, ACC_PAD_COLS], f32)
        # memset on DVE (idle at startup): on Pool it would sit ahead of the
        # rhs SWDGE descriptor generations and delay the first theta by ~4us
        nc.vector.memset(acc[:], 0.0)
        lhs_t = outer.tile([KR, 128], bf16)
        # lhs goes out on Pool's DGE so rhs chunk 0 is SP's first issue --
        # the two tiny transfers then overlap instead of pacing 625ns apart
        nc.gpsimd.dma_start(lhs_t[:], d_lhs[:])
        iden = outer.tile([128, 128], bf16)
        # iden's DMA is issued inside the core body after the first input
        # chunk: it is first read only at the first scatter (~10us in) and
        # must not delay rhs chunk 0 on the DMA engines.

        if not single_core:
            dram = octx.enter_context(
                tc.tile_pool(name="dram", bufs=1, space="DRAM"))
            b_in = dram.tile([128, ACC_COLS], f32)

        def emit_core_body(core):
            """Returns nothing; flushes final acc columns eagerly into the
            reduce input (SPMD) or the output (single-core estimate)."""
            info = cores[core]
            q = info["q"]
            strips = info["strips"]

            if single_core:
                flush_dst = d_full
            else:
                flush_dst = b_in[:]

            with ExitStack() as ctx:
                rhsp = ctx.enter_context(
                    tc.tile_pool(name=f"rhs{core}", bufs=1))
                wap = ctx.enter_context(tc.tile_pool(name=f"wap{core}", bufs=5))
                sp = ctx.enter_context(tc.tile_pool(name=f"sp{core}", bufs=3))
                vp = ctx.enter_context(
                    tc.tile_pool(name=f"vp{core}", bufs=SCATTER_LAG + 2))
                thp = ctx.enter_context(
                    tc.tile_pool(name=f"th{core}", bufs=2, space="PSUM"))
                stp = ctx.enter_context(
                    tc.tile_pool(name=f"st{core}", bufs=2, space="PSUM"))

                # PE warm-up: ~2us of throwaway matmuls on zeros so the PE
                # p-state ramp is past its low tier when rhs chunk 0 lands
                wt = stp.tile([128, STRIP_COLS], f32, tag="strip")
                for _ in range(18):
                    nc.tensor.matmul(
                        wt[:, :128], wz[:], wz[:], start=True, stop=True)

                # strip state machine across batches
                strip_iter = iter(strips)
                cur = next(strip_iter)
                cur_tile = None
                flushed = 0  # acc cols already written out
                NB = BATCH * GCOLS  # 1161

                def flush_to(limit, min_cols=FLUSH_MIN, sp=False):
                    """DMA-out final acc cols [flushed, limit).  Issued from
                    the otherwise-idle Pool queue mid-run (a flush's sem-wait
                    on pending strip evacs must not delay the SP-issued input
                    DMAs); drain-time flushes take SP's HWDGE, whose issue
                    path is ~1us faster than Pool's SWDGE generation."""
                    nonlocal flushed
                    lim = min(limit, ACC_COLS)
                    if lim - flushed >= min_cols:
                        eng = nc.scalar if sp else nc.gpsimd
                        eng.dma_start(
                            flush_dst[:, flushed:lim], acc[:, flushed:lim])
                        flushed = lim

                def emit_scatter(g0, t_v, ng=BATCH, voff=0):
                    nonlocal cur, cur_tile
                    for j in range(ng):
                        g = g0 + j
                        if g >= GPC:  # zero-amplitude pad grain
                            continue
                        # open new strip?
                        if g > cur[2]:
                            # evacuate finished strip (covered span)
                            w = cur[3] - cur[0]
                            nc.vector.tensor_add(
                                acc[:, cur[0]:cur[0] + w],
                                cur_tile[:, :w],
                                acc[:, cur[0]:cur[0] + w],
                            )
                            cur = next(strip_iter)
                            cur_tile = None
                            # flush eagerly in small pieces near the end so
                            # the drain-time flushes are tiny; at the last
                            # strip force everything below its base out
                            near_end = g >= GPC - 6 * BATCH
                            flush_to(cur[0],
                                     min_cols=(1 if cur is strips[-1] else
                                               64 if near_end else FLUSH_MIN),
                                     sp=near_end)
                        first = cur_tile is None
                        if first:
                            cur_tile = stp.tile(
                                [128, STRIP_COLS], f32, tag="strip")
                        off = int(q[g]) - cur[0]
                        last = g == cur[2]
                        nc.tensor.matmul(
                            cur_tile[:, off:off + GCOLS],
                            iden[:],
                            t_v[:, voff + j * GCOLS:voff + (j + 1) * GCOLS],
                            start=first, stop=last,
                        )

                # software pipeline: scatter runs SCATTER_LAG batches behind
                # the theta->sin->window chain so PE's in-order queue never
                # blocks upcoming theta matmuls on a v that was only just
                # produced (the ACT->DVE->v chain is ~1.2us deep).
                from collections import deque
                pending = deque()  # (g0, t_v) awaiting scatter

                # Input DMA routing: all rhs chunks are issued upfront from
                # the Pool queue (SWDGE) so they are never queued behind a wa
                # pool-slot wait -- if they were, the Tile scheduler's own
                # pipeline sim would see theta matmuls blocked on rhs and lock
                # a scatter-before-theta PE order that stalls ACT at runtime.
                # wa chunks stream on SP, self-paced by their pool slots.
                n_chunks = (n_batches + DMA_B - 1) // DMA_B
                rhs_all = rhsp.tile([KR, ncols], bf16, tag="rhs")
                wa_tiles = []

                def emit_rhs_dma(k):
                    col0 = k * DMA_B * NB
                    W2 = min(DMA_B, n_batches - k * DMA_B) * NB
                    # chunk 0 from SP (fast HWDGE, ahead of wa0 on the DMA
                    # engines -- it gates the first theta); the rest from
                    # Pool/SWDGE so they never sit behind a wa slot wait
                    eng = nc.sync if k == 0 else nc.gpsimd
                    eng.dma_start(
                        rhs_all[:, col0:col0 + W2], d_rhs[:, col0:col0 + W2])

                def emit_wa_dma(k):
                    col0 = k * DMA_B * NB
                    W2 = min(DMA_B, n_batches - k * DMA_B) * NB
                    t = wap.tile([128, DMA_B * NB], bf16, tag="wa")
                    nc.sync.dma_start(t[:, :W2], d_wa[:, col0:col0 + W2])
                    wa_tiles.append(t)

                for k in range(n_chunks):
                    emit_rhs_dma(k)
                for k in range(n_chunks):
                    emit_wa_dma(k)
                    if k == 0:
                        nc.sync.dma_start(iden[:], d_iden[:])

                for b2 in range(0, n_batches, DMA_B):
                    nb2 = min(DMA_B, n_batches - b2)
                    t_wa2 = wa_tiles[b2 // DMA_B]

                    for b in range(b2, b2 + nb2):
                        part = (b - b2) * NB
                        g0 = b * BATCH
                        t_rhs = rhs_all[:, b * NB:(b + 1) * NB]

                        # the first and last batches run at GPB-grain
                        # granularity: the first so ACT starts ~1us sooner
                        # (one theta matmul instead of three gates it), the
                        # last so the end-of-program ACT->mul->scatter chain
                        # is 1/3 the depth; fully-dummy sub-units are skipped
                        lastb = b == n_batches - 1
                        split = lastb or b == 0
                        nsub = -(-(GPC - g0) // GPB) if lastb else 3
                        nsub = max(1, min(3, nsub))

                        th = thp.tile([128, 3 * 512], f32, tag="th")
                        # priority-bias the theta matmuls ahead of the
                        # previous batch's scatter matmuls so PE's in-order
                        # dispatch never blocks them behind a v-wait.
                        with tc.high_priority(offset=16):
                            for m in range(nsub):
                                sl = slice(
                                    m * GPB * GCOLS, (m + 1) * GPB * GCOLS)
                                nc.tensor.matmul(
                                    th[:, m * 512: m * 512 + GPB * GCOLS],
                                    lhs_t[:],
                                    t_rhs[:, sl],
                                    start=True, stop=True,
                                )
                        t_s = sp.tile([128, NB], bf16, tag="s")
                        t_v = vp.tile([128, NB], bf16, tag="v")
                        if not split:
                            th3 = th[:].rearrange(
                                "p (b x) -> p b x", b=3)[:, :, :GPB * GCOLS]
                            s3 = t_s[:].rearrange("p (b x) -> p b x", b=3)
                            nc.scalar.activation(s3, th3, AF.Sin, scale=TWO_PI)
                            nc.vector.tensor_mul(
                                t_v[:], t_s[:], t_wa2[:, part:part + NB])
                            pending.append((g0, t_v, BATCH, 0))
                            if len(pending) > SCATTER_LAG:
                                emit_scatter(*pending.popleft())
                        else:
                            U = GPB * GCOLS
                            for m in range(nsub):
                                nc.scalar.activation(
                                    t_s[:, m * U:(m + 1) * U],
                                    th[:, m * 512: m * 512 + U],
                                    AF.Sin, scale=TWO_PI)
                                nc.vector.tensor_mul(
                                    t_v[:, m * U:(m + 1) * U],
                                    t_s[:, m * U:(m + 1) * U],
                                    t_wa2[:, part + m * U:part + (m + 1) * U])
                                pending.append((g0 + m * GPB, t_v, GPB, m * U))
                                if len(pending) > SCATTER_LAG:
                                    emit_scatter(*pending.popleft())
                while pending:
                    emit_scatter(*pending.popleft())
                # final (small) strip: evacuate and flush the remainder
                base, w = cur[0], cur[3] - cur[0]
                nc.vector.tensor_add(
                    acc[:, base:base + w], cur_tile[:, :w],
                    acc[:, base:base + w])
                nc.sync.dma_start(
                    flush_dst[:, flushed:ACC_COLS],
                    acc[:, flushed:ACC_COLS])
                del wa_tiles[:]

        if single_core:
            d_full = nc.dram_tensor(
                "full", [128, ACC_COLS], f32, kind="ExternalOutput").ap()
            emit_core_body(0)
        else:
            pid = nc.partition_id()
            for core in range(N_CORES):
                with tc.If(pid == core):
                    emit_core_body(core)

            # ---- shared epilog: reduce, normalize, output ----
            b_rs = dram.tile([16, ACC_COLS], f32)
            nc.gpsimd.collective_compute(
                "ReduceScatter",
                mybir.AluOpType.add,
                replica_groups=[list(range(N_CORES))],
                ins=[b_in[:].opt()],
                outs=[b_rs[:].opt()],
            )
            red = outer.tile([128, 512], f32)
            nc.sync.dma_start(
                red[:],
                b_rs[:].rearrange("a b -> (a b)").rearrange(
                    "(p c) -> p c", p=128))

            # sum of squares of the local chunk
            scr = outer.tile([128, 512], f32)
            sqcol = outer.tile([128, 1], f32)
            nc.scalar.activation(scr[:], red[:], AF.Square, accum_out=sqcol[:])
            ones = outer.tile([128, 128], f32)
            nc.vector.memset(ones[:], 1.0)
            psq = octx.enter_context(tc.tile_pool(name="psq", bufs=1, space="PSUM"))
            ps_s = psq.tile([1, 128], f32)
            nc.tensor.matmul(ps_s[:], sqcol[:], ones[:], start=True, stop=True)
            ssq = outer.tile([1, 128], f32)
            nc.vector.tensor_copy(ssq[:], ps_s[:])

            b_s1 = dram.tile([1, 128], f32)
            b_s2 = dram.tile([1, 128], f32)
            nc.sync.dma_start(b_s1[:], ssq[:])
            nc.gpsimd.collective_compute(
                "AllReduce",
                mybir.AluOpType.add,
                replica_groups=[list(range(N_CORES))],
                ins=[b_s1[:].opt()],
                outs=[b_s2[:].opt()],
            )
            gsq = outer.tile([1, 1], f32)
            nc.sync.dma_start(gsq[:], b_s2[:, 0:1])

            # rscale = rsqrt(gsq) with one Newton refinement
            nrm = outer.tile([1, 1], f32)
            nc.scalar.activation(nrm[:], gsq[:], AF.Sqrt)
            z0 = outer.tile([1, 1], f32)
            nc.vector.reciprocal(z0[:], nrm[:])
            z2 = outer.tile([1, 1], f32)
            nc.vector.tensor_mul(z2[:], z0[:], z0[:])
            t2 = outer.tile([1, 1], f32)
            nc.vector.tensor_mul(t2[:], z2[:], gsq[:])
            t3 = outer.tile([1, 1], f32)
            nc.vector.tensor_scalar(t3[:], t2[:], -0.5, 1.5, ALU.mult, ALU.add)
            z1 = outer.tile([1, 1], f32)
            nc.vector.tensor_mul(z1[:], z0[:], t3[:])

            # broadcast to 128 partitions via DRAM bounce
            b_z = dram.tile([1, 1], f32)
            nc.sync.dma_start(b_z[:], z1[:])
            zb = outer.tile([128, 1], f32)
            bz_ap = b_z[:]
            bcast = bass.AP(tensor=bz_ap.tensor, offset=bz_ap.offset,
                            ap=[[0, 128], [1, 1]])
            nc.sync.dma_start(zb[:], bcast)

            outt = outer.tile([128, 512], f32)
            nc.vector.tensor_scalar(outt[:], red[:], zb[:], None, ALU.mult)
            nc.sync.dma_start(
                d_out.rearrange("(p c) -> p c", p=128), outt[:])

    nc.compile()
    return nc


def estimate_hw_time_ns(theta_density, theta_slope, f0_freqs_hz, onsets):
    """Cost-model (TimelineSim) estimate of one core's execution, ns.

    Single-core variant: core 0's synthesis+scatter+evac plus the 2MB
    accumulator DMA-out (standing in for the ReduceScatter contribution).
    """
    from concourse.timeline_sim import TimelineSim

    cores, meta = _host_prep(theta_density, theta_slope, f0_freqs_hz, onsets)
    nc = _build_program(cores, meta, single_core=True)
    ts = TimelineSim(nc)
    ts.simulate()
    return float(ts.time)


def kernel(theta_density, theta_slope, f0_freqs_hz, onsets):
    import ml_dtypes
    from concourse import bass_utils

    cores, meta = _host_prep(theta_density, theta_slope, f0_freqs_hz, onsets)
    nc = _build_program(cores, meta)

    iden = np.eye(128, dtype=np.float32).astype(ml_dtypes.bfloat16)
    in_maps = []
    for c in range(N_CORES):
        info = cores[c]
        in_maps.append(
            dict(
                lhs=meta["lhs"],
                rhs=info["rhs"],
                wa=info["wa"],
                iden=iden,
            )
        )
    res = bass_utils.run_bass_kernel_spmd(
        nc, in_maps, core_ids=list(range(N_CORES)))

    X = np.zeros((ACC_COLS, 128), dtype=np.float32)
    for c in range(N_CORES):
        chunk = res.results[c]["out"].reshape(16, ACC_COLS)
        X[:, 16 * c:16 * (c + 1)] = chunk.T
    return X.reshape(-1).astype(np.float32)


if __name__ == "__main__":
    rng = np.random.default_rng(0)
    out = kernel(
        np.float32(0.5), np.float32(0.3),
        np.exp(rng.uniform(np.log(F0_MIN), np.log(F0_MAX), N_GRAINS)).astype(np.float32),
        rng.integers(0, N_SAMPLES - GRAIN_N, N_GRAINS).astype(np.int32),
    )
    print(out.shape, out[:8], np.linalg.norm(out))

